# revision 12
# baseline (speedup 1.0000x reference)
"""GAT layer (DGL GATConv + BatchNorm + ELU + residual) on 8 Trainium2 cores.

Single-launch design (dst-sharded graph parallel):
  - Destination nodes load-balanced into 98 blocks x 128 slots per core.
  - Feat table [100352, 128] fp16 (256B rows) built distributed: each core
    computes its 1/8 slice (x @ W) and an AllGather replicates it.
  - Edge gathers use gpsimd dma_gather (batched SWDGE): one instruction per
    (supergroup of 4 blocks, src quadrant) fetches up to 2560 rows, killing
    the per-instruction ~1us Pool overhead that dominated the per-tile
    indirect-DMA baseline. int16 indices => 4 table windows of 25088 rows.
  - el per edge = reduce(feat * attn_l) on DVE; er per edge via one-hot
    matmuls (dst-slot one-hot built from iota is_equal).
  - Messages scaled by w = exp(leaky_relu(el+er)) in fp16; scatter-reduce
    into PSUM via S^T @ [w*feat | w] on the PE per 128-edge tile.
  - BatchNorm batch stats: per-core partial sums -> 1KB AllReduce -> affine
    fold + ELU + residual applied in-kernel (h stays in SBUF).
"""
import sys
sys.path.insert(0, "/opt/trn_rl_repo")
import numpy as np

import concourse.bass as bass
import concourse.bacc as bacc
import concourse.mybir as mybir
import concourse.tile as tile
from concourse.bass_utils import run_bass_kernel_spmd

F32 = mybir.dt.float32
F16 = mybir.dt.float16
I16 = mybir.dt.int16

N = 100000
E = 1600000
IN_DIM = 128
H = 8
D = 16
HD = 128
NCORES = 8
SLOTS = 12544             # dst slots per core (98 blocks x 128)
NBLK = 98
NTOT = NCORES * SLOTS     # 100352 padded node count
QROWS = NTOT // 4         # 25088 rows per int16-addressable table window
TPC = 5                   # tiles per (block, quadrant) cell
QCAP = TPC * 128          # 640 edge slots per cell
TILES = NBLK * 4 * TPC    # 1960 tiles per core
EDGES_PAD = TILES * 128   # 250880 edge slots per core
SGS = [(s * 4, min(4, NBLK - s * 4)) for s in range((NBLK + 3) // 4)]
NEG_SLOPE = 0.2
EPS = 1e-5

LAST_EXEC_NS = [0, 0]

_cache = {}


def _build():
    nc = bacc.Bacc("TRN2", target_bir_lowering=False, debug=False,
                   num_devices=NCORES)
    xTs = nc.dram_tensor("xTs", [128, SLOTS], F16, kind="ExternalInput")
    xTp = nc.dram_tensor("xTp", [128, SLOTS], F16, kind="ExternalInput")
    xsd = nc.dram_tensor("xs", [SLOTS, HD], F16, kind="ExternalInput")
    Wd = nc.dram_tensor("W", [IN_DIM, HD], F32, kind="ExternalInput")
    amd = nc.dram_tensor("am", [HD, 2 * H], F32, kind="ExternalInput")
    gbbd = nc.dram_tensor("gbb", [128, 3], F32, kind="ExternalInput")
    atrd = nc.dram_tensor("attnrep", [128, 128], F16, kind="ExternalInput")
    iotad = nc.dram_tensor("iota16", [128, 128], F16, kind="ExternalInput")
    iotacd = nc.dram_tensor("iotac", [128, 1], F32, kind="ExternalInput")
    idxd = nc.dram_tensor("idx16", [128, EDGES_PAD // 16], I16,
                          kind="ExternalInput")
    dsd = nc.dram_tensor("dslot16", [128, TILES], F16, kind="ExternalInput")
    drd = nc.dram_tensor("drow", [1, EDGES_PAD], F16, kind="ExternalInput")

    out_nm = nc.dram_tensor("out_nm", [SLOTS, HD], F32, kind="ExternalOutput")
    table = nc.dram_tensor("table", [NTOT, HD], F16)
    tsrc = nc.dram_tensor("tsrc", [SLOTS, HD], F16)
    stat_in = nc.dram_tensor("stat_in", [128, 2], F32)
    stat_out = nc.dram_tensor("stat_out", [128, 2], F32)

    RG = [list(range(NCORES))]

    with tile.TileContext(nc) as tc:
        with (
            tc.tile_pool(name="const", bufs=1) as constp,
            tc.tile_pool(name="pa_sb", bufs=6) as pa_sb,
            tc.tile_pool(name="row4", bufs=4) as rowp,
            tc.tile_pool(name="big", bufs=1) as bigp,
        ):
            # ---- constants ----
            iota_row = constp.tile([128, 128], F16)
            nc.sync.dma_start(out=iota_row[:], in_=iotad[:])
            iota_col = constp.tile([128, 1], F32)
            nc.sync.dma_start(out=iota_col[:], in_=iotacd[:])
            attn_rep = constp.tile([128, 128], F16)
            nc.sync.dma_start(out=attn_rep[:], in_=atrd[:])
            gbb_sb = constp.tile([128, 3], F32)
            nc.sync.dma_start(out=gbb_sb[:], in_=gbbd[:])
            dslot_sb = constp.tile([128, TILES], F16)
            nc.sync.dma_start(out=dslot_sb[:], in_=dsd[:])
            ones_row = constp.tile([1, 128], F16)
            nc.vector.memset(ones_row[:], 1.0)
            ones_row32 = constp.tile([1, 128], F32)
            nc.vector.memset(ones_row32[:], 1.0)
            ones_col16 = constp.tile([128, 1], F16)
            nc.vector.memset(ones_col16[:], 1.0)

            from concourse.masks import make_identity
            ident = constp.tile([128, 128], F32)
            make_identity(nc, ident[:])

            # ---- Wh = [W | W@almat | W@armat] fp16 [128, 144] ----
            pa_scope = tc.tile_pool(name="pa_ps", bufs=4, space="PSUM")
            pa_ps = pa_scope.__enter__()
            W_sb = constp.tile([128, HD], F32)
            nc.sync.dma_start(out=W_sb[:], in_=Wd[:])
            am_sb = constp.tile([128, 2 * H], F32)
            nc.sync.dma_start(out=am_sb[:], in_=amd[:])
            wt_ps = pa_ps.tile([128, 128], F32, tag="pa")
            nc.tensor.transpose(out=wt_ps[:], in_=W_sb[:], identity=ident[:])
            WT_sb = constp.tile([128, 128], F32)
            nc.vector.tensor_copy(out=WT_sb[:], in_=wt_ps[:])
            wlr_ps = pa_ps.tile([128, 2 * H], F32, tag="pa")
            nc.tensor.matmul(out=wlr_ps[:], lhsT=WT_sb[:], rhs=am_sb[:],
                             start=True, stop=True)
            Wh = constp.tile([128, HD + 2 * H], F16)
            nc.vector.tensor_copy(out=Wh[:, 0:HD], in_=W_sb[:])
            nc.vector.tensor_copy(out=Wh[:, HD:HD + 2 * H], in_=wlr_ps[:])

            # ---- residual x rows (slot-major) into SBUF ----
            xs_sb = bigp.tile([128, NBLK * HD], F16)
            nc.scalar.dma_start(
                out=xs_sb[:].rearrange("p (f c) -> p f c", c=HD),
                in_=xsd[:].rearrange("(f p) c -> p f c", f=NBLK))

            # ---- phase A: own table slice (x@W), er for own slots ----
            er_sb = bigp.tile([128, NBLK * H], F16)
            g4 = [(i * 4, min(4, NBLK - i * 4)) for i in range((NBLK + 3) // 4)]
            for (t0, nt) in g4:
                x4 = pa_sb.tile([128, 512], F16, tag="x4")
                nc.scalar.dma_start(out=x4[:, :nt * 128],
                                    in_=xTs[:, t0 * 128:(t0 + nt) * 128])
                xp4 = pa_sb.tile([128, 512], F16, tag="xp4")
                nc.sync.dma_start(out=xp4[:, :nt * 128],
                                  in_=xTp[:, t0 * 128:(t0 + nt) * 128])
                row4 = rowp.tile([128, 4 * HD], F16, tag="row4")
                er4_ps = pa_ps.tile([128, 4 * H], F32, tag="er4")
                for k in range(nt):
                    ps = pa_ps.tile([128, HD], F32, tag="pa")
                    nc.tensor.matmul(out=ps[:], lhsT=x4[:, k * 128:(k + 1) * 128],
                                     rhs=Wh[:, 0:HD], start=True, stop=True)
                    if k % 2 == 0:
                        nc.vector.tensor_copy(out=row4[:, k * HD:(k + 1) * HD],
                                              in_=ps[:])
                    else:
                        nc.scalar.activation(row4[:, k * HD:(k + 1) * HD],
                                             ps[:],
                                             mybir.ActivationFunctionType.Copy)
                    nc.tensor.matmul(out=er4_ps[:, k * H:(k + 1) * H],
                                     lhsT=xp4[:, k * 128:(k + 1) * 128],
                                     rhs=Wh[:, HD + H:HD + 2 * H],
                                     start=True, stop=True)
                nc.vector.tensor_copy(out=er_sb[:, t0 * H:(t0 + nt) * H],
                                      in_=er4_ps[:, 0:nt * H])
                nc.sync.dma_start(
                    out=tsrc[t0 * 128:(t0 + nt) * 128, :].rearrange(
                        "(f p) c -> p f c", f=nt),
                    in_=row4[:, 0:nt * HD].rearrange("p (f c) -> p f c", c=HD))
            pa_scope.__exit__(None, None, None)

            # ---- AllGather the table ----
            nc.gpsimd.collective_compute(
                kind="AllGather", op=mybir.AluOpType.bypass,
                replica_groups=RG, ins=[tsrc[:]], outs=[table[:]])

            # ---- phase B ----
            hall = bigp.tile([128, NBLK * HD], F32)

            sg_scope = tc.tile_pool(name="sg_ps", bufs=2, space="PSUM")
            sg_ps = sg_scope.__enter__()
            erp_scope = tc.tile_pool(name="erp_ps", bufs=1, space="PSUM")
            erp_psp = erp_scope.__enter__()
            psb_scope = tc.tile_pool(name="psb_ps", bufs=1, space="PSUM")
            psbp = psb_scope.__enter__()
            st_scope = tc.tile_pool(name="st_ps", bufs=1, space="PSUM")
            stat_ps = st_scope.__enter__()
            stat2_ps = stat_ps.tile([128, 2], F32)
            s1_ps = stat2_ps[:, 0:1]
            s2_ps = stat2_ps[:, 1:2]

            with (
                tc.tile_pool(name="gp", bufs=2) as gp,
                tc.tile_pool(name="gsp", bufs=2) as gsp,
                tc.tile_pool(name="idxp", bufs=2) as idxp,
                tc.tile_pool(name="drp", bufs=2) as drp,
                tc.tile_pool(name="st4p", bufs=2) as st4p,
                tc.tile_pool(name="ssp", bufs=2) as ssp,
                tc.tile_pool(name="tmpp", bufs=2) as tmpp,
                tc.tile_pool(name="elp", bufs=3) as elp,
                tc.tile_pool(name="wp", bufs=3) as wp,
                tc.tile_pool(name="finp", bufs=4) as finp,
            ):
                idx_col = 0   # running column offset into idxd (16 idxs/col)
                tile_col = 0  # running global tile index
                edge_off = 0  # running edge-slot offset (drow)
                gb_done = 0   # blocks finalized
                for (b0, nb) in SGS:
                    nt = nb * TPC              # tiles per (sg, q)
                    ne = nt * 128              # edge slots per (sg, q)
                    scols = ne // 16
                    idx_sb = idxp.tile([128, 4 * 160], I16, tag="idx")
                    nc.sync.dma_start(out=idx_sb[:, 0:4 * scols],
                                      in_=idxd[:, idx_col:idx_col + 4 * scols])
                    idx_col += 4 * scols
                    psbs = [psbp.tile([128, HD + H], F32, tag=f"psb{j}",
                                      name=f"psb{j}")
                            for j in range(nb)]
                    for q in range(4):
                        # --- batched gather of feat rows ---
                        g = gp.tile([128, 4 * TPC * 128], F16, tag="g")
                        nc.gpsimd.dma_gather(
                            g[:, 0:ne].rearrange("p (t c) -> p t c", c=128),
                            table[q * QROWS:(q + 1) * QROWS, :],
                            idx_sb[:, q * scols:(q + 1) * scols],
                            ne, ne, 128, single_packet=False)
                        # --- el = reduce(feat * attn_l) ---
                        tmp = tmpp.tile([128, 4 * TPC * 128], F16, tag="tmp")
                        nc.vector.tensor_tensor(
                            out=tmp[:, 0:ne].rearrange("p (t h d) -> p t h d",
                                                       h=H, d=D),
                            in0=g[:, 0:ne].rearrange("p (t h d) -> p t h d",
                                                     h=H, d=D),
                            in1=attn_rep[:].rearrange("p (o h d) -> p o h d",
                                                      o=1, h=H)
                                .to_broadcast([128, nt, H, D]),
                            op=mybir.AluOpType.mult)
                        el = elp.tile([128, 4 * TPC * H], F32, tag="el")
                        nc.vector.tensor_reduce(
                            out=el[:, 0:nt * H],
                            in_=tmp[:, 0:ne].rearrange("p (t h d) -> p t h d",
                                                       h=H, d=D),
                            axis=mybir.AxisListType.X,
                            op=mybir.AluOpType.add)
                        # --- st4 one-hot [slot, edge] for er expansion ---
                        dr = drp.tile([1, 4 * TPC * 128], F16, tag="dr")
                        nc.sync.dma_start(out=dr[:, 0:ne],
                                          in_=drd[:, edge_off:edge_off + ne])
                        edge_off += ne
                        st4 = st4p.tile([128, 4 * TPC * 128], F16, tag="st4")
                        for o in range(0, ne, 512):
                            cw = min(512, ne - o)
                            dtp = sg_ps.tile([128, 512], F32, tag="dtp")
                            nc.tensor.matmul(out=dtp[:, :cw], lhsT=ones_row[:],
                                             rhs=dr[:, o:o + cw],
                                             start=True, stop=True)
                            nc.vector.tensor_scalar(
                                out=st4[:, o:o + cw], in0=dtp[:, :cw],
                                scalar1=iota_col[:], scalar2=None,
                                op0=mybir.AluOpType.is_equal)
                        # --- er per edge via one-hot matmuls ---
                        erp = erp_psp.tile([128, 4 * TPC * H], F32, tag="erp")
                        for t in range(nt):
                            gb = b0 + t // TPC
                            nc.tensor.matmul(
                                out=erp[:, t * H:(t + 1) * H],
                                lhsT=st4[:, t * 128:(t + 1) * 128],
                                rhs=er_sb[:, gb * H:(gb + 1) * H],
                                start=True, stop=True)
                        # --- w = exp(leaky_relu(el + er)) ---
                        wb = wp.tile([128, 4 * TPC * H], F32, tag="wb")
                        nc.vector.tensor_tensor(out=wb[:, 0:nt * H],
                                                in0=el[:, 0:nt * H],
                                                in1=erp[:, 0:nt * H],
                                                op=mybir.AluOpType.add)
                        w5 = wp.tile([128, 4 * TPC * H], F32, tag="w5")
                        nc.vector.tensor_scalar(out=w5[:, 0:nt * H],
                                                in0=wb[:, 0:nt * H],
                                                scalar1=NEG_SLOPE, scalar2=None,
                                                op0=mybir.AluOpType.mult)
                        nc.vector.tensor_tensor(out=wb[:, 0:nt * H],
                                                in0=wb[:, 0:nt * H],
                                                in1=w5[:, 0:nt * H],
                                                op=mybir.AluOpType.max)
                        nc.scalar.activation(wb[:, 0:nt * H], wb[:, 0:nt * H],
                                             mybir.ActivationFunctionType.Exp)
                        # --- gs = [w*feat | w] fp16 ---
                        gs = gsp.tile([128, 4 * TPC * (HD + H)], F16, tag="gs")
                        gs_v = gs[:, 0:nt * (HD + H)].rearrange(
                            "p (t c) -> p t c", c=HD + H)
                        nc.scalar.activation(
                            gs_v[:, :, HD:HD + H],
                            wb[:, 0:nt * H],
                            mybir.ActivationFunctionType.Copy)
                        w16b = (gs_v[:, :, HD:HD + H]
                                .rearrange("p t (h o) -> p t h o", o=1)
                                .to_broadcast([128, nt, H, D]))
                        nc.vector.tensor_tensor(
                            out=gs_v[:, :, 0:HD].rearrange(
                                "p t (h d) -> p t h d", d=D),
                            in0=g[:, 0:ne].rearrange("p (t h d) -> p t h d",
                                                     h=H, d=D),
                            in1=w16b,
                            op=mybir.AluOpType.mult)
                        # --- s one-hot [edge, slot] + scatter matmuls ---
                        s_sb = ssp.tile([128, 4 * TPC * 128], F16, tag="s")
                        nc.vector.tensor_tensor(
                            out=s_sb[:, 0:ne].rearrange("p (t c) -> p t c",
                                                        c=128),
                            in0=iota_row[:].rearrange("p (o c) -> p o c", o=1)
                                .to_broadcast([128, nt, 128]),
                            in1=dslot_sb[:, tile_col + q * nt:
                                         tile_col + (q + 1) * nt]
                                .rearrange("p (t o) -> p t o", o=1)
                                .to_broadcast([128, nt, 128]),
                            op=mybir.AluOpType.is_equal)
                        for t in range(nt):
                            j = t // TPC
                            t5 = t % TPC
                            nc.tensor.matmul(
                                out=psbs[j][:],
                                lhsT=s_sb[:, t * 128:(t + 1) * 128],
                                rhs=gs[:, t * (HD + H):(t + 1) * (HD + H)],
                                start=(q == 0 and t5 == 0),
                                stop=(q == 3 and t5 == TPC - 1))
                    tile_col += 4 * nt
                    # --- finalize blocks of this supergroup ---
                    for j in range(nb):
                        gb = b0 + j
                        psb = psbs[j]
                        ssum = finp.tile([128, H], F32, tag="ssum")
                        nc.vector.tensor_scalar(out=ssum[:],
                                                in0=psb[:, HD:HD + H],
                                                scalar1=1e-30, scalar2=None,
                                                op0=mybir.AluOpType.add)
                        rec = finp.tile([128, H], F32, tag="rec")
                        nc.vector.reciprocal(out=rec[:], in_=ssum[:])
                        rec_b = (rec[:].rearrange("p (h o) -> p h o", o=1)
                                 .to_broadcast([128, H, D]))
                        hslice = hall[:, gb * HD:(gb + 1) * HD]
                        nc.vector.tensor_tensor(
                            out=hslice.rearrange("p (h d) -> p h d", d=D),
                            in0=psb[:, 0:HD].rearrange("p (h d) -> p h d", d=D),
                            in1=rec_b, op=mybir.AluOpType.mult)
                        h16 = finp.tile([128, HD], F16, tag="h16")
                        nc.vector.tensor_copy(out=h16[:], in_=hslice)
                        sq16 = finp.tile([128, HD], F16, tag="sq16")
                        nc.scalar.activation(sq16[:], hslice,
                                             mybir.ActivationFunctionType.Square)
                        nc.tensor.matmul(out=s1_ps, lhsT=h16[:],
                                         rhs=ones_col16[:],
                                         start=(gb == 0), stop=(gb == NBLK - 1))
                        nc.tensor.matmul(out=s2_ps, lhsT=sq16[:],
                                         rhs=ones_col16[:],
                                         start=(gb == 0), stop=(gb == NBLK - 1))
                    gb_done += nb

            # ---- BN stats AllReduce + affine fold ----
            stat_sb = constp.tile([128, 2], F32)
            nc.vector.tensor_copy(out=stat_sb[:], in_=stat2_ps[:])
            nc.sync.dma_start(out=stat_in[:], in_=stat_sb[:])
            st_scope.__exit__(None, None, None)
            psb_scope.__exit__(None, None, None)
            erp_scope.__exit__(None, None, None)
            sg_scope.__exit__(None, None, None)
            nc.gpsimd.collective_compute(
                kind="AllReduce", op=mybir.AluOpType.add,
                replica_groups=RG, ins=[stat_in[:]], outs=[stat_out[:]])
            str_sb = constp.tile([128, 2], F32)
            nc.sync.dma_start(out=str_sb[:], in_=stat_out[:])

            ac2 = constp.tile([128, 2], F32)
            m1 = constp.tile([128, 1], F32)
            nc.vector.tensor_scalar(out=m1[:], in0=str_sb[:, 0:1],
                                    scalar1=1.0 / N, scalar2=None,
                                    op0=mybir.AluOpType.mult)
            m2 = constp.tile([128, 1], F32)
            nc.vector.tensor_scalar(out=m2[:], in0=str_sb[:, 1:2],
                                    scalar1=1.0 / N, scalar2=None,
                                    op0=mybir.AluOpType.mult)
            vv = constp.tile([128, 1], F32)
            nc.vector.tensor_tensor(out=vv[:], in0=m1[:], in1=m1[:],
                                    op=mybir.AluOpType.mult)
            nc.vector.tensor_tensor(out=vv[:], in0=m2[:], in1=vv[:],
                                    op=mybir.AluOpType.subtract)
            nc.vector.tensor_scalar(out=vv[:], in0=vv[:],
                                    scalar1=EPS, scalar2=None,
                                    op0=mybir.AluOpType.add)
            nc.scalar.activation(vv[:], vv[:],
                                 mybir.ActivationFunctionType.Sqrt)
            rsq = constp.tile([128, 1], F32)
            nc.vector.reciprocal(out=rsq[:], in_=vv[:])
            # a = gamma * rsqrt(var+eps)
            nc.vector.tensor_tensor(out=ac2[:, 0:1], in0=gbb_sb[:, 0:1],
                                    in1=rsq[:], op=mybir.AluOpType.mult)
            # c = beta - a * (mu + bias)
            mu = constp.tile([128, 1], F32)
            nc.vector.tensor_tensor(out=mu[:], in0=m1[:], in1=gbb_sb[:, 2:3],
                                    op=mybir.AluOpType.add)
            amu = constp.tile([128, 1], F32)
            nc.vector.tensor_tensor(out=amu[:], in0=ac2[:, 0:1], in1=mu[:],
                                    op=mybir.AluOpType.mult)
            nc.vector.tensor_tensor(out=ac2[:, 1:2], in0=gbb_sb[:, 1:2],
                                    in1=amu[:], op=mybir.AluOpType.subtract)

            fin_scope = tc.tile_pool(name="fin_ps", bufs=1, space="PSUM")
            fin_ps = fin_scope.__enter__()
            aT_ps = fin_ps.tile([1, 128], F32, tag="aT")
            nc.tensor.transpose(out=aT_ps[:], in_=ac2[:, 0:1],
                                identity=ident[:])
            cT_ps = fin_ps.tile([1, 128], F32, tag="cT")
            nc.tensor.transpose(out=cT_ps[:], in_=ac2[:, 1:2],
                                identity=ident[:])
            acT = constp.tile([1, 256], F32)
            nc.vector.tensor_copy(out=acT[:, 0:128], in_=aT_ps[:])
            nc.vector.tensor_copy(out=acT[:, 128:256], in_=cT_ps[:])
            ac_ps = fin_ps.tile([128, 256], F32, tag="AC")
            nc.tensor.matmul(out=ac_ps[:, 0:128], lhsT=ones_row32[:],
                             rhs=acT[0:1, 0:128], start=True, stop=True)
            nc.tensor.matmul(out=ac_ps[:, 128:256], lhsT=ones_row32[:],
                             rhs=acT[0:1, 128:256], start=True, stop=True)
            AC = constp.tile([128, 256], F32)
            nc.vector.tensor_copy(out=AC[:], in_=ac_ps[:])

            # ---- finalize: h2 = A*h + C ; ELU ; + x ; store ----
            with tc.tile_pool(name="fin2", bufs=3) as fin2p:
                FB = 8  # blocks per finalize group
                for f0 in range(0, NBLK, FB):
                    fb = min(FB, NBLK - f0)
                    w = fb * HD
                    h2 = fin2p.tile([128, FB * HD], F32, tag="h2")
                    nc.vector.tensor_tensor(
                        out=h2[:, 0:w].rearrange("p (f c) -> p f c", c=HD),
                        in0=hall[:, f0 * HD:(f0 + fb) * HD]
                            .rearrange("p (f c) -> p f c", c=HD),
                        in1=AC[:, 0:HD].rearrange("p (o c) -> p o c", o=1)
                            .to_broadcast([128, fb, HD]),
                        op=mybir.AluOpType.mult)
                    nc.vector.tensor_tensor(
                        out=h2[:, 0:w].rearrange("p (f c) -> p f c", c=HD),
                        in0=h2[:, 0:w].rearrange("p (f c) -> p f c", c=HD),
                        in1=AC[:, HD:256].rearrange("p (o c) -> p o c", o=1)
                            .to_broadcast([128, fb, HD]),
                        op=mybir.AluOpType.add)
                    m = fin2p.tile([128, FB * HD], F32, tag="m")
                    nc.vector.tensor_scalar(out=m[:, 0:w], in0=h2[:, 0:w],
                                            scalar1=0.0, scalar2=None,
                                            op0=mybir.AluOpType.min)
                    nc.scalar.activation(m[:, 0:w], m[:, 0:w],
                                         mybir.ActivationFunctionType.Exp)
                    nc.vector.tensor_scalar(out=m[:, 0:w], in0=m[:, 0:w],
                                            scalar1=-1.0, scalar2=None,
                                            op0=mybir.AluOpType.add)
                    nc.vector.tensor_tensor(out=h2[:, 0:w], in0=h2[:, 0:w],
                                            in1=m[:, 0:w],
                                            op=mybir.AluOpType.max)
                    nc.vector.tensor_tensor(
                        out=h2[:, 0:w], in0=h2[:, 0:w],
                        in1=xs_sb[:, f0 * HD:(f0 + fb) * HD],
                        op=mybir.AluOpType.add)
                    nc.sync.dma_start(
                        out=out_nm[f0 * 128:(f0 + fb) * 128, :].rearrange(
                            "(f p) c -> p f c", f=fb),
                        in_=h2[:, 0:w].rearrange("p (f c) -> p f c", c=HD))
            fin_scope.__exit__(None, None, None)

    nc.compile()
    return nc


def _host_prep(x, src, dst):
    """Shard + balance + pad. Returns per-core index arrays and mappings."""
    import heapq
    per_core = []
    equad_all = src // QROWS
    for c in range(NCORES):
        lo = c * SLOTS
        hi = min((c + 1) * SLOTS, N)
        nodes_c = hi - lo
        m = (dst >= lo) & (dst < hi)
        e_src = src[m].astype(np.int64)
        e_dst = (dst[m] - lo).astype(np.int64)
        e_q = equad_all[m].astype(np.int64)
        deg = np.bincount(e_dst, minlength=nodes_c)
        cq = np.zeros((nodes_c, 4), np.int64)
        np.add.at(cq, (e_dst, e_q), 1)
        order = np.argsort(-deg, kind="stable")
        # greedy balance with per-quadrant caps
        heap = [(0, b) for b in range(NBLK)]
        heapq.heapify(heap)
        slots_used = np.zeros(NBLK, np.int64)
        qload = np.zeros((NBLK, 4), np.int64)
        blk_of = np.empty(nodes_c, np.int64)
        slot_of = np.empty(nodes_c, np.int64)
        for v in order:
            spill = []
            while True:
                load, b = heapq.heappop(heap)
                if slots_used[b] < 128 and np.all(qload[b] + cq[v] <= QCAP):
                    break
                spill.append((load, b))
            blk_of[v] = b
            slot_of[v] = slots_used[b]
            slots_used[b] += 1
            qload[b] += cq[v]
            heapq.heappush(heap, (load + int(deg[v]), b))
            for it in spill:
                heapq.heappush(heap, it)
        # place edges into fixed (block, quadrant, 640) cells
        eb = blk_of[e_dst]
        eslot = slot_of[e_dst]
        key = eb * 4 + e_q
        cnt = np.bincount(key, minlength=NBLK * 4)
        assert cnt.max() <= QCAP, f"cell overflow {cnt.max()} > {QCAP}"
        eorder = np.argsort(key, kind="stable")
        offs = np.zeros(NBLK * 4 + 1, np.int64)
        np.cumsum(cnt, out=offs[1:])
        within = np.arange(len(eb)) - offs[key[eorder]]
        srcq_arr = np.zeros((NBLK, 4, QCAP), np.int16)
        slot_arr = np.full((NBLK, 4, QCAP), 300.0, np.float32)
        ko = key[eorder]
        srcq_arr[ko // 4, ko % 4, within] = (
            e_src[eorder] - e_q[eorder] * QROWS).astype(np.int16)
        slot_arr[ko // 4, ko % 4, within] = eslot[eorder]
        node_of_slot = np.full(SLOTS, -1, np.int64)
        node_of_slot[blk_of * 128 + slot_of] = np.arange(nodes_c) + lo
        per_core.append((srcq_arr, slot_arr, node_of_slot))
    return per_core


def _wrap16(vals):
    """int16 gather index layout: (p, s) = vals[s*16 + p%16], 8 replicas."""
    n = len(vals)
    w = vals.reshape(n // 16, 16).T.astype(np.int16)
    return np.tile(w, (8, 1))


def kernel(x, src, dst, W, attn_l, attn_r, bias, gamma, beta):
    global LAST_EXEC_NS
    x = np.asarray(x, np.float32)
    src = np.asarray(src, np.int64)
    dst = np.asarray(dst, np.int64)
    W = np.asarray(W, np.float32)
    attn_l = np.asarray(attn_l, np.float32)
    attn_r = np.asarray(attn_r, np.float32)
    bias = np.asarray(bias, np.float32)
    gamma = np.asarray(gamma, np.float32)
    beta = np.asarray(beta, np.float32)

    if "nc" not in _cache:
        _cache["nc"] = _build()
    nc = _cache["nc"]

    per_core = _host_prep(x, src, dst)

    xT = np.zeros((128, NTOT), np.float16)
    xT[:, :N] = x.T.astype(np.float16)
    am = np.zeros((HD, 2 * H), np.float32)
    for h in range(H):
        am[h * D:(h + 1) * D, h] = attn_l[h]
        am[h * D:(h + 1) * D, H + h] = attn_r[h]
    gbb = np.stack([gamma, beta, bias], axis=1).astype(np.float32)
    attnrep = np.tile(attn_l.reshape(1, H * D), (128, 1)).astype(np.float16)
    iota16 = np.tile(np.arange(128, dtype=np.float16), (128, 1))
    iotac = np.arange(128, dtype=np.float32).reshape(128, 1)

    in_maps = []
    for c in range(NCORES):
        srcq_arr, slot_arr, node_of_slot = per_core[c]
        # gather idx stream + dslot/drow in (sg, q, b, t, p) enumeration
        idx_cols = []
        ds_cols = []
        dr_vals = []
        for (b0, nb) in SGS:
            for q in range(4):
                flat_idx = srcq_arr[b0:b0 + nb, q, :].reshape(-1)
                flat_slot = slot_arr[b0:b0 + nb, q, :].reshape(-1)
                idx_cols.append(_wrap16(flat_idx))
                ds_cols.append(flat_slot.reshape(nb * TPC, 128).T)
                dr_vals.append(flat_slot)
        idx16 = np.concatenate(idx_cols, axis=1).astype(np.int16)
        dslot16 = np.concatenate(ds_cols, axis=1).astype(np.float16)
        drow = np.concatenate(dr_vals).reshape(1, EDGES_PAD).astype(np.float16)

        real = node_of_slot >= 0
        xTp = np.zeros((128, SLOTS), np.float16)
        xTp[:, real] = x[node_of_slot[real]].T.astype(np.float16)
        xs = np.zeros((SLOTS, HD), np.float16)
        xs[real] = x[node_of_slot[real]].astype(np.float16)

        in_maps.append({
            "xTs": np.ascontiguousarray(xT[:, c * SLOTS:(c + 1) * SLOTS]),
            "xTp": xTp, "xs": xs, "W": W, "am": am, "gbb": gbb,
            "attnrep": attnrep, "iota16": iota16, "iotac": iotac,
            "idx16": idx16, "dslot16": dslot16, "drow": drow,
        })

    res = run_bass_kernel_spmd(nc, in_maps, list(range(NCORES)),
                               **_trace_kwargs())
    LAST_EXEC_NS = [res.exec_time_ns or 0, 0]

    out = np.zeros((N, IN_DIM), np.float32)
    for c in range(NCORES):
        node_of_slot = per_core[c][2]
        real = node_of_slot >= 0
        om = res.results[c]["out_nm"]  # [SLOTS, HD]
        out[node_of_slot[real]] = om[real]
    return out


def _trace_kwargs():
    import os
    if os.environ.get("GAT_TRACE", "0") == "1":
        return {"trace": True}
    return {}


# revision 14
# speedup vs baseline: 1.0310x; 1.0310x over previous
"""GAT layer (DGL GATConv + BatchNorm + ELU + residual) on 8 Trainium2 cores.

Single-launch design (dst-sharded graph parallel):
  - Destination nodes load-balanced into 98 blocks x 128 slots per core.
  - Feat table [100352, 128] fp16 (256B rows) built distributed: each core
    computes its 1/8 slice (x @ W) and an AllGather replicates it.
  - Edge gathers use gpsimd dma_gather (batched SWDGE): one instruction per
    (supergroup of 4 blocks, src quadrant) fetches up to 2560 rows, killing
    the per-instruction ~1us Pool overhead that dominated the per-tile
    indirect-DMA baseline. int16 indices => 4 table windows of 25088 rows.
  - el per edge = reduce(feat * attn_l) on DVE; er per edge via one-hot
    matmuls (dst-slot one-hot built from iota is_equal).
  - Messages scaled by w = exp(leaky_relu(el+er)) in fp16; scatter-reduce
    into PSUM via S^T @ [w*feat | w] on the PE per 128-edge tile.
  - BatchNorm batch stats: per-core partial sums -> 1KB AllReduce -> affine
    fold + ELU + residual applied in-kernel (h stays in SBUF).
"""
import sys
sys.path.insert(0, "/opt/trn_rl_repo")
import numpy as np

import concourse.bass as bass
import concourse.bacc as bacc
import concourse.mybir as mybir
import concourse.tile as tile
from concourse.bass_utils import run_bass_kernel_spmd

F32 = mybir.dt.float32
F16 = mybir.dt.float16
I16 = mybir.dt.int16

N = 100000
E = 1600000
IN_DIM = 128
H = 8
D = 16
HD = 128
NCORES = 8
SLOTS = 12544             # dst slots per core (98 blocks x 128)
NBLK = 98
NTOT = NCORES * SLOTS     # 100352 padded node count
QROWS = NTOT // 4         # 25088 rows per int16-addressable table window
TPC = 5                   # tiles per (block, quadrant) cell
QCAP = TPC * 128          # 640 edge slots per cell
TILES = NBLK * 4 * TPC    # 1960 tiles per core
EDGES_PAD = TILES * 128   # 250880 edge slots per core
SGS = [(s * 4, min(4, NBLK - s * 4)) for s in range((NBLK + 3) // 4)]
NEG_SLOPE = 0.2
EPS = 1e-5

LAST_EXEC_NS = [0, 0]

_cache = {}


def _build():
    nc = bacc.Bacc("TRN2", target_bir_lowering=False, debug=False,
                   num_devices=NCORES, num_swdge_queues=4)
    xTs = nc.dram_tensor("xTs", [128, SLOTS], F16, kind="ExternalInput")
    xTp = nc.dram_tensor("xTp", [128, SLOTS], F16, kind="ExternalInput")
    xsd = nc.dram_tensor("xs", [SLOTS, HD], F16, kind="ExternalInput")
    Wd = nc.dram_tensor("W", [IN_DIM, HD], F32, kind="ExternalInput")
    amd = nc.dram_tensor("am", [HD, 2 * H], F32, kind="ExternalInput")
    gbbd = nc.dram_tensor("gbb", [128, 3], F32, kind="ExternalInput")
    atrd = nc.dram_tensor("attnrep", [128, 128], F16, kind="ExternalInput")
    iotad = nc.dram_tensor("iota16", [128, 128], F16, kind="ExternalInput")
    iotacd = nc.dram_tensor("iotac", [128, 1], F32, kind="ExternalInput")
    idxd = nc.dram_tensor("idx16", [128, EDGES_PAD // 16], I16,
                          kind="ExternalInput")
    dsd = nc.dram_tensor("dslot16", [128, TILES], F16, kind="ExternalInput")
    drd = nc.dram_tensor("drow", [1, EDGES_PAD], F16, kind="ExternalInput")

    out_nm = nc.dram_tensor("out_nm", [SLOTS, HD], F32, kind="ExternalOutput")
    table = nc.dram_tensor("table", [NTOT, HD], F16)
    tsrc = nc.dram_tensor("tsrc", [SLOTS, HD], F16)
    stat_in = nc.dram_tensor("stat_in", [128, 2], F32)
    stat_out = nc.dram_tensor("stat_out", [128, 2], F32)

    RG = [list(range(NCORES))]

    with tile.TileContext(nc) as tc:
        with (
            tc.tile_pool(name="const", bufs=1) as constp,
            tc.tile_pool(name="pa_sb", bufs=6) as pa_sb,
            tc.tile_pool(name="row4", bufs=4) as rowp,
            tc.tile_pool(name="big", bufs=1) as bigp,
        ):
            # ---- constants ----
            iota_row = constp.tile([128, 128], F16)
            nc.sync.dma_start(out=iota_row[:], in_=iotad[:])
            iota_col = constp.tile([128, 1], F32)
            nc.sync.dma_start(out=iota_col[:], in_=iotacd[:])
            attn_rep = constp.tile([128, 128], F16)
            nc.sync.dma_start(out=attn_rep[:], in_=atrd[:])
            gbb_sb = constp.tile([128, 3], F32)
            nc.sync.dma_start(out=gbb_sb[:], in_=gbbd[:])
            dslot_sb = constp.tile([128, TILES], F16)
            nc.sync.dma_start(out=dslot_sb[:], in_=dsd[:])
            ones_row = constp.tile([1, 128], F16)
            nc.vector.memset(ones_row[:], 1.0)
            ones_row32 = constp.tile([1, 128], F32)
            nc.vector.memset(ones_row32[:], 1.0)
            ones_col16 = constp.tile([128, 1], F16)
            nc.vector.memset(ones_col16[:], 1.0)

            from concourse.masks import make_identity
            ident = constp.tile([128, 128], F32)
            make_identity(nc, ident[:])

            # ---- Wh = [W | W@almat | W@armat] fp16 [128, 144] ----
            pa_scope = tc.tile_pool(name="pa_ps", bufs=4, space="PSUM")
            pa_ps = pa_scope.__enter__()
            W_sb = constp.tile([128, HD], F32)
            nc.sync.dma_start(out=W_sb[:], in_=Wd[:])
            am_sb = constp.tile([128, 2 * H], F32)
            nc.sync.dma_start(out=am_sb[:], in_=amd[:])
            wt_ps = pa_ps.tile([128, 128], F32, tag="pa")
            nc.tensor.transpose(out=wt_ps[:], in_=W_sb[:], identity=ident[:])
            WT_sb = constp.tile([128, 128], F32)
            nc.vector.tensor_copy(out=WT_sb[:], in_=wt_ps[:])
            wlr_ps = pa_ps.tile([128, 2 * H], F32, tag="pa")
            nc.tensor.matmul(out=wlr_ps[:], lhsT=WT_sb[:], rhs=am_sb[:],
                             start=True, stop=True)
            Wh = constp.tile([128, HD + 2 * H], F16)
            nc.vector.tensor_copy(out=Wh[:, 0:HD], in_=W_sb[:])
            nc.vector.tensor_copy(out=Wh[:, HD:HD + 2 * H], in_=wlr_ps[:])

            # ---- residual x rows (slot-major) into SBUF ----
            xs_sb = bigp.tile([128, NBLK * HD], F16)
            nc.scalar.dma_start(
                out=xs_sb[:].rearrange("p (f c) -> p f c", c=HD),
                in_=xsd[:].rearrange("(f p) c -> p f c", f=NBLK))

            # ---- phase A: own table slice (x@W), er for own slots ----
            er_sb = bigp.tile([128, NBLK * H], F16)
            g4 = [(i * 4, min(4, NBLK - i * 4)) for i in range((NBLK + 3) // 4)]
            for (t0, nt) in g4:
                x4 = pa_sb.tile([128, 512], F16, tag="x4")
                nc.scalar.dma_start(out=x4[:, :nt * 128],
                                    in_=xTs[:, t0 * 128:(t0 + nt) * 128])
                xp4 = pa_sb.tile([128, 512], F16, tag="xp4")
                nc.sync.dma_start(out=xp4[:, :nt * 128],
                                  in_=xTp[:, t0 * 128:(t0 + nt) * 128])
                row4 = rowp.tile([128, 4 * HD], F16, tag="row4")
                er4_ps = pa_ps.tile([128, 4 * H], F32, tag="er4")
                for k in range(nt):
                    ps = pa_ps.tile([128, HD], F32, tag="pa")
                    nc.tensor.matmul(out=ps[:], lhsT=x4[:, k * 128:(k + 1) * 128],
                                     rhs=Wh[:, 0:HD], start=True, stop=True)
                    if k % 2 == 0:
                        nc.vector.tensor_copy(out=row4[:, k * HD:(k + 1) * HD],
                                              in_=ps[:])
                    else:
                        nc.scalar.activation(row4[:, k * HD:(k + 1) * HD],
                                             ps[:],
                                             mybir.ActivationFunctionType.Copy)
                    nc.tensor.matmul(out=er4_ps[:, k * H:(k + 1) * H],
                                     lhsT=xp4[:, k * 128:(k + 1) * 128],
                                     rhs=Wh[:, HD + H:HD + 2 * H],
                                     start=True, stop=True)
                nc.vector.tensor_copy(out=er_sb[:, t0 * H:(t0 + nt) * H],
                                      in_=er4_ps[:, 0:nt * H])
                nc.sync.dma_start(
                    out=tsrc[t0 * 128:(t0 + nt) * 128, :].rearrange(
                        "(f p) c -> p f c", f=nt),
                    in_=row4[:, 0:nt * HD].rearrange("p (f c) -> p f c", c=HD))
            pa_scope.__exit__(None, None, None)

            # ---- AllGather the table ----
            nc.gpsimd.collective_compute(
                kind="AllGather", op=mybir.AluOpType.bypass,
                replica_groups=RG, ins=[tsrc[:]], outs=[table[:]])

            # ---- phase B ----
            hall = bigp.tile([128, NBLK * HD], F32)

            sg_scope = tc.tile_pool(name="sg_ps", bufs=2, space="PSUM")
            sg_ps = sg_scope.__enter__()
            erp_scope = tc.tile_pool(name="erp_ps", bufs=1, space="PSUM")
            erp_psp = erp_scope.__enter__()
            psb_scope = tc.tile_pool(name="psb_ps", bufs=1, space="PSUM")
            psbp = psb_scope.__enter__()
            st_scope = tc.tile_pool(name="st_ps", bufs=1, space="PSUM")
            stat_ps = st_scope.__enter__()
            stat2_ps = stat_ps.tile([128, 2], F32)
            s1_ps = stat2_ps[:, 0:1]
            s2_ps = stat2_ps[:, 1:2]

            with (
                tc.tile_pool(name="gp", bufs=2) as gp,
                tc.tile_pool(name="gsp", bufs=2) as gsp,
                tc.tile_pool(name="idxp", bufs=2) as idxp,
                tc.tile_pool(name="drp", bufs=2) as drp,
                tc.tile_pool(name="st4p", bufs=2) as st4p,
                tc.tile_pool(name="ssp", bufs=2) as ssp,
                tc.tile_pool(name="tmpp", bufs=2) as tmpp,
                tc.tile_pool(name="elp", bufs=3) as elp,
                tc.tile_pool(name="wp", bufs=3) as wp,
                tc.tile_pool(name="finp", bufs=4) as finp,
            ):
                idx_col = 0   # running column offset into idxd (16 idxs/col)
                tile_col = 0  # running global tile index
                edge_off = 0  # running edge-slot offset (drow)
                gb_done = 0   # blocks finalized
                for (b0, nb) in SGS:
                    nt = nb * TPC              # tiles per (sg, q)
                    ne = nt * 128              # edge slots per (sg, q)
                    scols = ne // 16
                    idx_sb = idxp.tile([128, 4 * 160], I16, tag="idx")
                    nc.sync.dma_start(out=idx_sb[:, 0:4 * scols],
                                      in_=idxd[:, idx_col:idx_col + 4 * scols])
                    idx_col += 4 * scols
                    psbs = [psbp.tile([128, HD + H], F32, tag=f"psb{j}",
                                      name=f"psb{j}")
                            for j in range(nb)]
                    for q in range(4):
                        # --- batched gather of feat rows ---
                        g = gp.tile([128, 4 * TPC * 128], F16, tag="g")
                        nc.gpsimd.dma_gather(
                            g[:, 0:ne].rearrange("p (t c) -> p t c", c=128),
                            table[q * QROWS:(q + 1) * QROWS, :],
                            idx_sb[:, q * scols:(q + 1) * scols],
                            ne, ne, 128, single_packet=False, queue_num=q)
                        # --- el = reduce(feat * attn_l) ---
                        tmp = tmpp.tile([128, 4 * TPC * 128], F16, tag="tmp")
                        nc.vector.tensor_tensor(
                            out=tmp[:, 0:ne].rearrange("p (t h d) -> p t h d",
                                                       h=H, d=D),
                            in0=g[:, 0:ne].rearrange("p (t h d) -> p t h d",
                                                     h=H, d=D),
                            in1=attn_rep[:].rearrange("p (o h d) -> p o h d",
                                                      o=1, h=H)
                                .to_broadcast([128, nt, H, D]),
                            op=mybir.AluOpType.mult)
                        el = elp.tile([128, 4 * TPC * H], F32, tag="el")
                        nc.vector.tensor_reduce(
                            out=el[:, 0:nt * H],
                            in_=tmp[:, 0:ne].rearrange("p (t h d) -> p t h d",
                                                       h=H, d=D),
                            axis=mybir.AxisListType.X,
                            op=mybir.AluOpType.add)
                        # --- st4 one-hot [slot, edge] for er expansion ---
                        dr = drp.tile([1, 4 * TPC * 128], F16, tag="dr")
                        nc.sync.dma_start(out=dr[:, 0:ne],
                                          in_=drd[:, edge_off:edge_off + ne])
                        edge_off += ne
                        st4 = st4p.tile([128, 4 * TPC * 128], F16, tag="st4")
                        for o in range(0, ne, 512):
                            cw = min(512, ne - o)
                            dtp = sg_ps.tile([128, 512], F32, tag="dtp")
                            nc.tensor.matmul(out=dtp[:, :cw], lhsT=ones_row[:],
                                             rhs=dr[:, o:o + cw],
                                             start=True, stop=True)
                            nc.vector.tensor_scalar(
                                out=st4[:, o:o + cw], in0=dtp[:, :cw],
                                scalar1=iota_col[:], scalar2=None,
                                op0=mybir.AluOpType.is_equal)
                        # --- er per edge via one-hot matmuls ---
                        erp = erp_psp.tile([128, 4 * TPC * H], F32, tag="erp")
                        for t in range(nt):
                            gb = b0 + t // TPC
                            nc.tensor.matmul(
                                out=erp[:, t * H:(t + 1) * H],
                                lhsT=st4[:, t * 128:(t + 1) * 128],
                                rhs=er_sb[:, gb * H:(gb + 1) * H],
                                start=True, stop=True)
                        # --- w = exp(leaky_relu(el + er)) ---
                        wb = wp.tile([128, 4 * TPC * H], F32, tag="wb")
                        nc.vector.tensor_tensor(out=wb[:, 0:nt * H],
                                                in0=el[:, 0:nt * H],
                                                in1=erp[:, 0:nt * H],
                                                op=mybir.AluOpType.add)
                        w5 = wp.tile([128, 4 * TPC * H], F32, tag="w5")
                        nc.vector.tensor_scalar(out=w5[:, 0:nt * H],
                                                in0=wb[:, 0:nt * H],
                                                scalar1=NEG_SLOPE, scalar2=None,
                                                op0=mybir.AluOpType.mult)
                        nc.vector.tensor_tensor(out=wb[:, 0:nt * H],
                                                in0=wb[:, 0:nt * H],
                                                in1=w5[:, 0:nt * H],
                                                op=mybir.AluOpType.max)
                        nc.scalar.activation(wb[:, 0:nt * H], wb[:, 0:nt * H],
                                             mybir.ActivationFunctionType.Exp)
                        # --- gs = [w*feat | w] fp16 ---
                        gs = gsp.tile([128, 4 * TPC * (HD + H)], F16, tag="gs")
                        gs_v = gs[:, 0:nt * (HD + H)].rearrange(
                            "p (t c) -> p t c", c=HD + H)
                        nc.scalar.activation(
                            gs_v[:, :, HD:HD + H],
                            wb[:, 0:nt * H],
                            mybir.ActivationFunctionType.Copy)
                        w16b = (gs_v[:, :, HD:HD + H]
                                .rearrange("p t (h o) -> p t h o", o=1)
                                .to_broadcast([128, nt, H, D]))
                        nc.vector.tensor_tensor(
                            out=gs_v[:, :, 0:HD].rearrange(
                                "p t (h d) -> p t h d", d=D),
                            in0=g[:, 0:ne].rearrange("p (t h d) -> p t h d",
                                                     h=H, d=D),
                            in1=w16b,
                            op=mybir.AluOpType.mult)
                        # --- s one-hot [edge, slot] + scatter matmuls ---
                        s_sb = ssp.tile([128, 4 * TPC * 128], F16, tag="s")
                        nc.vector.tensor_tensor(
                            out=s_sb[:, 0:ne].rearrange("p (t c) -> p t c",
                                                        c=128),
                            in0=iota_row[:].rearrange("p (o c) -> p o c", o=1)
                                .to_broadcast([128, nt, 128]),
                            in1=dslot_sb[:, tile_col + q * nt:
                                         tile_col + (q + 1) * nt]
                                .rearrange("p (t o) -> p t o", o=1)
                                .to_broadcast([128, nt, 128]),
                            op=mybir.AluOpType.is_equal)
                        for t in range(nt):
                            j = t // TPC
                            t5 = t % TPC
                            nc.tensor.matmul(
                                out=psbs[j][:],
                                lhsT=s_sb[:, t * 128:(t + 1) * 128],
                                rhs=gs[:, t * (HD + H):(t + 1) * (HD + H)],
                                start=(q == 0 and t5 == 0),
                                stop=(q == 3 and t5 == TPC - 1))
                    tile_col += 4 * nt
                    # --- finalize blocks of this supergroup ---
                    for j in range(nb):
                        gb = b0 + j
                        psb = psbs[j]
                        ssum = finp.tile([128, H], F32, tag="ssum")
                        nc.vector.tensor_scalar(out=ssum[:],
                                                in0=psb[:, HD:HD + H],
                                                scalar1=1e-30, scalar2=None,
                                                op0=mybir.AluOpType.add)
                        rec = finp.tile([128, H], F32, tag="rec")
                        nc.vector.reciprocal(out=rec[:], in_=ssum[:])
                        rec_b = (rec[:].rearrange("p (h o) -> p h o", o=1)
                                 .to_broadcast([128, H, D]))
                        hslice = hall[:, gb * HD:(gb + 1) * HD]
                        nc.vector.tensor_tensor(
                            out=hslice.rearrange("p (h d) -> p h d", d=D),
                            in0=psb[:, 0:HD].rearrange("p (h d) -> p h d", d=D),
                            in1=rec_b, op=mybir.AluOpType.mult)
                        h16 = finp.tile([128, HD], F16, tag="h16")
                        nc.vector.tensor_copy(out=h16[:], in_=hslice)
                        sq16 = finp.tile([128, HD], F16, tag="sq16")
                        nc.scalar.activation(sq16[:], hslice,
                                             mybir.ActivationFunctionType.Square)
                        nc.tensor.matmul(out=s1_ps, lhsT=h16[:],
                                         rhs=ones_col16[:],
                                         start=(gb == 0), stop=(gb == NBLK - 1))
                        nc.tensor.matmul(out=s2_ps, lhsT=sq16[:],
                                         rhs=ones_col16[:],
                                         start=(gb == 0), stop=(gb == NBLK - 1))
                    gb_done += nb

            # ---- BN stats AllReduce + affine fold ----
            stat_sb = constp.tile([128, 2], F32)
            nc.vector.tensor_copy(out=stat_sb[:], in_=stat2_ps[:])
            nc.sync.dma_start(out=stat_in[:], in_=stat_sb[:])
            st_scope.__exit__(None, None, None)
            psb_scope.__exit__(None, None, None)
            erp_scope.__exit__(None, None, None)
            sg_scope.__exit__(None, None, None)
            nc.gpsimd.collective_compute(
                kind="AllReduce", op=mybir.AluOpType.add,
                replica_groups=RG, ins=[stat_in[:]], outs=[stat_out[:]])
            str_sb = constp.tile([128, 2], F32)
            nc.sync.dma_start(out=str_sb[:], in_=stat_out[:])

            ac2 = constp.tile([128, 2], F32)
            m1 = constp.tile([128, 1], F32)
            nc.vector.tensor_scalar(out=m1[:], in0=str_sb[:, 0:1],
                                    scalar1=1.0 / N, scalar2=None,
                                    op0=mybir.AluOpType.mult)
            m2 = constp.tile([128, 1], F32)
            nc.vector.tensor_scalar(out=m2[:], in0=str_sb[:, 1:2],
                                    scalar1=1.0 / N, scalar2=None,
                                    op0=mybir.AluOpType.mult)
            vv = constp.tile([128, 1], F32)
            nc.vector.tensor_tensor(out=vv[:], in0=m1[:], in1=m1[:],
                                    op=mybir.AluOpType.mult)
            nc.vector.tensor_tensor(out=vv[:], in0=m2[:], in1=vv[:],
                                    op=mybir.AluOpType.subtract)
            nc.vector.tensor_scalar(out=vv[:], in0=vv[:],
                                    scalar1=EPS, scalar2=None,
                                    op0=mybir.AluOpType.add)
            nc.scalar.activation(vv[:], vv[:],
                                 mybir.ActivationFunctionType.Sqrt)
            rsq = constp.tile([128, 1], F32)
            nc.vector.reciprocal(out=rsq[:], in_=vv[:])
            # a = gamma * rsqrt(var+eps)
            nc.vector.tensor_tensor(out=ac2[:, 0:1], in0=gbb_sb[:, 0:1],
                                    in1=rsq[:], op=mybir.AluOpType.mult)
            # c = beta - a * (mu + bias)
            mu = constp.tile([128, 1], F32)
            nc.vector.tensor_tensor(out=mu[:], in0=m1[:], in1=gbb_sb[:, 2:3],
                                    op=mybir.AluOpType.add)
            amu = constp.tile([128, 1], F32)
            nc.vector.tensor_tensor(out=amu[:], in0=ac2[:, 0:1], in1=mu[:],
                                    op=mybir.AluOpType.mult)
            nc.vector.tensor_tensor(out=ac2[:, 1:2], in0=gbb_sb[:, 1:2],
                                    in1=amu[:], op=mybir.AluOpType.subtract)

            fin_scope = tc.tile_pool(name="fin_ps", bufs=1, space="PSUM")
            fin_ps = fin_scope.__enter__()
            aT_ps = fin_ps.tile([1, 128], F32, tag="aT")
            nc.tensor.transpose(out=aT_ps[:], in_=ac2[:, 0:1],
                                identity=ident[:])
            cT_ps = fin_ps.tile([1, 128], F32, tag="cT")
            nc.tensor.transpose(out=cT_ps[:], in_=ac2[:, 1:2],
                                identity=ident[:])
            acT = constp.tile([1, 256], F32)
            nc.vector.tensor_copy(out=acT[:, 0:128], in_=aT_ps[:])
            nc.vector.tensor_copy(out=acT[:, 128:256], in_=cT_ps[:])
            ac_ps = fin_ps.tile([128, 256], F32, tag="AC")
            nc.tensor.matmul(out=ac_ps[:, 0:128], lhsT=ones_row32[:],
                             rhs=acT[0:1, 0:128], start=True, stop=True)
            nc.tensor.matmul(out=ac_ps[:, 128:256], lhsT=ones_row32[:],
                             rhs=acT[0:1, 128:256], start=True, stop=True)
            AC = constp.tile([128, 256], F32)
            nc.vector.tensor_copy(out=AC[:], in_=ac_ps[:])

            # ---- finalize: h2 = A*h + C ; ELU ; + x ; store ----
            with tc.tile_pool(name="fin2", bufs=3) as fin2p:
                FB = 8  # blocks per finalize group
                for f0 in range(0, NBLK, FB):
                    fb = min(FB, NBLK - f0)
                    w = fb * HD
                    h2 = fin2p.tile([128, FB * HD], F32, tag="h2")
                    nc.vector.tensor_tensor(
                        out=h2[:, 0:w].rearrange("p (f c) -> p f c", c=HD),
                        in0=hall[:, f0 * HD:(f0 + fb) * HD]
                            .rearrange("p (f c) -> p f c", c=HD),
                        in1=AC[:, 0:HD].rearrange("p (o c) -> p o c", o=1)
                            .to_broadcast([128, fb, HD]),
                        op=mybir.AluOpType.mult)
                    nc.vector.tensor_tensor(
                        out=h2[:, 0:w].rearrange("p (f c) -> p f c", c=HD),
                        in0=h2[:, 0:w].rearrange("p (f c) -> p f c", c=HD),
                        in1=AC[:, HD:256].rearrange("p (o c) -> p o c", o=1)
                            .to_broadcast([128, fb, HD]),
                        op=mybir.AluOpType.add)
                    m = fin2p.tile([128, FB * HD], F32, tag="m")
                    nc.vector.tensor_scalar(out=m[:, 0:w], in0=h2[:, 0:w],
                                            scalar1=0.0, scalar2=None,
                                            op0=mybir.AluOpType.min)
                    nc.scalar.activation(m[:, 0:w], m[:, 0:w],
                                         mybir.ActivationFunctionType.Exp)
                    nc.vector.tensor_scalar(out=m[:, 0:w], in0=m[:, 0:w],
                                            scalar1=-1.0, scalar2=None,
                                            op0=mybir.AluOpType.add)
                    nc.vector.tensor_tensor(out=h2[:, 0:w], in0=h2[:, 0:w],
                                            in1=m[:, 0:w],
                                            op=mybir.AluOpType.max)
                    nc.vector.tensor_tensor(
                        out=h2[:, 0:w], in0=h2[:, 0:w],
                        in1=xs_sb[:, f0 * HD:(f0 + fb) * HD],
                        op=mybir.AluOpType.add)
                    nc.sync.dma_start(
                        out=out_nm[f0 * 128:(f0 + fb) * 128, :].rearrange(
                            "(f p) c -> p f c", f=fb),
                        in_=h2[:, 0:w].rearrange("p (f c) -> p f c", c=HD))
            fin_scope.__exit__(None, None, None)

    nc.compile()
    return nc


def _host_prep(x, src, dst):
    """Shard + balance + pad. Returns per-core index arrays and mappings."""
    import heapq
    per_core = []
    equad_all = src // QROWS
    for c in range(NCORES):
        lo = c * SLOTS
        hi = min((c + 1) * SLOTS, N)
        nodes_c = hi - lo
        m = (dst >= lo) & (dst < hi)
        e_src = src[m].astype(np.int64)
        e_dst = (dst[m] - lo).astype(np.int64)
        e_q = equad_all[m].astype(np.int64)
        deg = np.bincount(e_dst, minlength=nodes_c)
        cq = np.zeros((nodes_c, 4), np.int64)
        np.add.at(cq, (e_dst, e_q), 1)
        order = np.argsort(-deg, kind="stable")
        # greedy balance with per-quadrant caps
        heap = [(0, b) for b in range(NBLK)]
        heapq.heapify(heap)
        slots_used = np.zeros(NBLK, np.int64)
        qload = np.zeros((NBLK, 4), np.int64)
        blk_of = np.empty(nodes_c, np.int64)
        slot_of = np.empty(nodes_c, np.int64)
        for v in order:
            spill = []
            while True:
                load, b = heapq.heappop(heap)
                if slots_used[b] < 128 and np.all(qload[b] + cq[v] <= QCAP):
                    break
                spill.append((load, b))
            blk_of[v] = b
            slot_of[v] = slots_used[b]
            slots_used[b] += 1
            qload[b] += cq[v]
            heapq.heappush(heap, (load + int(deg[v]), b))
            for it in spill:
                heapq.heappush(heap, it)
        # place edges into fixed (block, quadrant, 640) cells
        eb = blk_of[e_dst]
        eslot = slot_of[e_dst]
        key = eb * 4 + e_q
        cnt = np.bincount(key, minlength=NBLK * 4)
        assert cnt.max() <= QCAP, f"cell overflow {cnt.max()} > {QCAP}"
        eorder = np.argsort(key, kind="stable")
        offs = np.zeros(NBLK * 4 + 1, np.int64)
        np.cumsum(cnt, out=offs[1:])
        within = np.arange(len(eb)) - offs[key[eorder]]
        srcq_arr = np.zeros((NBLK, 4, QCAP), np.int16)
        slot_arr = np.full((NBLK, 4, QCAP), 300.0, np.float32)
        ko = key[eorder]
        srcq_arr[ko // 4, ko % 4, within] = (
            e_src[eorder] - e_q[eorder] * QROWS).astype(np.int16)
        slot_arr[ko // 4, ko % 4, within] = eslot[eorder]
        node_of_slot = np.full(SLOTS, -1, np.int64)
        node_of_slot[blk_of * 128 + slot_of] = np.arange(nodes_c) + lo
        per_core.append((srcq_arr, slot_arr, node_of_slot))
    return per_core


def _wrap16(vals):
    """int16 gather index layout: (p, s) = vals[s*16 + p%16], 8 replicas."""
    n = len(vals)
    w = vals.reshape(n // 16, 16).T.astype(np.int16)
    return np.tile(w, (8, 1))


def kernel(x, src, dst, W, attn_l, attn_r, bias, gamma, beta):
    global LAST_EXEC_NS
    x = np.asarray(x, np.float32)
    src = np.asarray(src, np.int64)
    dst = np.asarray(dst, np.int64)
    W = np.asarray(W, np.float32)
    attn_l = np.asarray(attn_l, np.float32)
    attn_r = np.asarray(attn_r, np.float32)
    bias = np.asarray(bias, np.float32)
    gamma = np.asarray(gamma, np.float32)
    beta = np.asarray(beta, np.float32)

    if "nc" not in _cache:
        _cache["nc"] = _build()
    nc = _cache["nc"]

    per_core = _host_prep(x, src, dst)

    xT = np.zeros((128, NTOT), np.float16)
    xT[:, :N] = x.T.astype(np.float16)
    am = np.zeros((HD, 2 * H), np.float32)
    for h in range(H):
        am[h * D:(h + 1) * D, h] = attn_l[h]
        am[h * D:(h + 1) * D, H + h] = attn_r[h]
    gbb = np.stack([gamma, beta, bias], axis=1).astype(np.float32)
    attnrep = np.tile(attn_l.reshape(1, H * D), (128, 1)).astype(np.float16)
    iota16 = np.tile(np.arange(128, dtype=np.float16), (128, 1))
    iotac = np.arange(128, dtype=np.float32).reshape(128, 1)

    in_maps = []
    for c in range(NCORES):
        srcq_arr, slot_arr, node_of_slot = per_core[c]
        # gather idx stream + dslot/drow in (sg, q, b, t, p) enumeration
        idx_cols = []
        ds_cols = []
        dr_vals = []
        for (b0, nb) in SGS:
            for q in range(4):
                flat_idx = srcq_arr[b0:b0 + nb, q, :].reshape(-1)
                flat_slot = slot_arr[b0:b0 + nb, q, :].reshape(-1)
                idx_cols.append(_wrap16(flat_idx))
                ds_cols.append(flat_slot.reshape(nb * TPC, 128).T)
                dr_vals.append(flat_slot)
        idx16 = np.concatenate(idx_cols, axis=1).astype(np.int16)
        dslot16 = np.concatenate(ds_cols, axis=1).astype(np.float16)
        drow = np.concatenate(dr_vals).reshape(1, EDGES_PAD).astype(np.float16)

        real = node_of_slot >= 0
        xTp = np.zeros((128, SLOTS), np.float16)
        xTp[:, real] = x[node_of_slot[real]].T.astype(np.float16)
        xs = np.zeros((SLOTS, HD), np.float16)
        xs[real] = x[node_of_slot[real]].astype(np.float16)

        in_maps.append({
            "xTs": np.ascontiguousarray(xT[:, c * SLOTS:(c + 1) * SLOTS]),
            "xTp": xTp, "xs": xs, "W": W, "am": am, "gbb": gbb,
            "attnrep": attnrep, "iota16": iota16, "iotac": iotac,
            "idx16": idx16, "dslot16": dslot16, "drow": drow,
        })

    res = run_bass_kernel_spmd(nc, in_maps, list(range(NCORES)),
                               **_trace_kwargs())
    LAST_EXEC_NS = [res.exec_time_ns or 0, 0]

    out = np.zeros((N, IN_DIM), np.float32)
    for c in range(NCORES):
        node_of_slot = per_core[c][2]
        real = node_of_slot >= 0
        om = res.results[c]["out_nm"]  # [SLOTS, HD]
        out[node_of_slot[real]] = om[real]
    return out


def _trace_kwargs():
    import os
    if os.environ.get("GAT_TRACE", "0") == "1":
        return {"trace": True}
    return {}


# revision 21
# speedup vs baseline: 1.2380x; 1.2007x over previous
"""GAT layer (DGL GATConv + BatchNorm + ELU + residual) on 8 Trainium2 cores.

Single-launch design (dst-sharded graph parallel):
  - Destination nodes load-balanced into 98 blocks x 128 slots per core.
  - Feat table [100352, 128] fp16 (256B rows) built distributed: each core
    computes its 1/8 slice (x @ W) and an AllGather replicates it.
  - Edge gathers use gpsimd dma_gather (batched SWDGE): one instruction per
    (supergroup of 4 blocks, src quadrant) fetches up to 2560 rows, killing
    the per-instruction ~1us Pool overhead that dominated the per-tile
    indirect-DMA baseline. int16 indices => 4 table windows of 25088 rows.
  - el per edge = reduce(feat * attn_l) on DVE; er per edge via one-hot
    matmuls (dst-slot one-hot built from iota is_equal).
  - Messages scaled by w = exp(leaky_relu(el+er)) in fp16; scatter-reduce
    into PSUM via S^T @ [w*feat | w] on the PE per 128-edge tile.
  - BatchNorm batch stats: per-core partial sums -> 1KB AllReduce -> affine
    fold + ELU + residual applied in-kernel (h stays in SBUF).
"""
import sys
sys.path.insert(0, "/opt/trn_rl_repo")
import numpy as np

import concourse.bass as bass
import concourse.bacc as bacc
import concourse.mybir as mybir
import concourse.tile as tile
from concourse.bass_utils import run_bass_kernel_spmd

F32 = mybir.dt.float32
F16 = mybir.dt.float16
I16 = mybir.dt.int16

N = 100000
E = 1600000
IN_DIM = 128
H = 8
D = 16
HD = 128
NCORES = 8
SLOTS = 12544             # dst slots per core (98 blocks x 128)
NBLK = 98
NTOT = NCORES * SLOTS     # 100352 padded node count
QROWS = NTOT // 4         # 25088 rows per int16-addressable table window
TPC = 5                   # tiles per (block, quadrant) cell
QCAP = TPC * 128          # 640 edge slots per cell
TILES = NBLK * 4 * TPC    # 1960 tiles per core
EDGES_PAD = TILES * 128   # 250880 edge slots per core
SGS = [(s * 4, min(4, NBLK - s * 4)) for s in range((NBLK + 3) // 4)]
NEG_SLOPE = 0.2
EPS = 1e-5

LAST_EXEC_NS = [0, 0]

_cache = {}


def _build():
    nc = bacc.Bacc("TRN2", target_bir_lowering=False, debug=False,
                   num_devices=NCORES, num_swdge_queues=4)
    xTs = nc.dram_tensor("xTs", [128, SLOTS], F16, kind="ExternalInput")
    xTp = nc.dram_tensor("xTp", [128, SLOTS], F16, kind="ExternalInput")
    xsd = nc.dram_tensor("xs", [SLOTS, HD], F16, kind="ExternalInput")
    Wd = nc.dram_tensor("W", [IN_DIM, HD], F32, kind="ExternalInput")
    amd = nc.dram_tensor("am", [HD, 2 * H], F32, kind="ExternalInput")
    gbbd = nc.dram_tensor("gbb", [128, 3], F32, kind="ExternalInput")
    atrd = nc.dram_tensor("attnrep", [128, 4 * TPC * 128], F16,
                          kind="ExternalInput")
    iotad = nc.dram_tensor("iota16", [128, 128], F16, kind="ExternalInput")
    iotacd = nc.dram_tensor("iotac", [128, 1], F32, kind="ExternalInput")
    idxd = nc.dram_tensor("idx16", [128, EDGES_PAD // 16], I16,
                          kind="ExternalInput")
    dsd = nc.dram_tensor("dslot16", [128, TILES], F16, kind="ExternalInput")
    drd = nc.dram_tensor("drow", [1, EDGES_PAD], F16, kind="ExternalInput")

    out_nm = nc.dram_tensor("out_nm", [SLOTS, HD], F32, kind="ExternalOutput")
    table = nc.dram_tensor("table", [NTOT, HD], F16)
    tsrc = nc.dram_tensor("tsrc", [SLOTS, HD], F16)
    stat_in = nc.dram_tensor("stat_in", [128, 2], F32)
    stat_out = nc.dram_tensor("stat_out", [128, 2], F32)

    RG = [list(range(NCORES))]

    with tile.TileContext(nc) as tc:
        with (
            tc.tile_pool(name="const", bufs=1) as constp,
            tc.tile_pool(name="pa_sb", bufs=6) as pa_sb,
            tc.tile_pool(name="row4", bufs=4) as rowp,
            tc.tile_pool(name="big", bufs=1) as bigp,
        ):
            # ---- constants ----
            iota_row = constp.tile([128, 128], F16)
            nc.sync.dma_start(out=iota_row[:], in_=iotad[:])
            iota_col = constp.tile([128, 1], F32)
            nc.sync.dma_start(out=iota_col[:], in_=iotacd[:])
            attn_rep = constp.tile([128, 4 * TPC * 128], F16)
            nc.sync.dma_start(out=attn_rep[:], in_=atrd[:])
            gbb_sb = constp.tile([128, 3], F32)
            nc.sync.dma_start(out=gbb_sb[:], in_=gbbd[:])
            dslot_sb = constp.tile([128, TILES], F16)
            nc.sync.dma_start(out=dslot_sb[:], in_=dsd[:])
            ones_row = constp.tile([1, 128], F16)
            nc.vector.memset(ones_row[:], 1.0)
            ones_row32 = constp.tile([1, 128], F32)
            nc.vector.memset(ones_row32[:], 1.0)
            ones_col16 = constp.tile([128, 1], F16)
            nc.vector.memset(ones_col16[:], 1.0)

            from concourse.masks import make_identity
            ident = constp.tile([128, 128], F32)
            make_identity(nc, ident[:])

            # ---- Wh = [W | W@almat | W@armat] fp16 [128, 144] ----
            pa_scope = tc.tile_pool(name="pa_ps", bufs=4, space="PSUM")
            pa_ps = pa_scope.__enter__()
            W_sb = constp.tile([128, HD], F32)
            nc.sync.dma_start(out=W_sb[:], in_=Wd[:])
            am_sb = constp.tile([128, 2 * H], F32)
            nc.sync.dma_start(out=am_sb[:], in_=amd[:])
            wt_ps = pa_ps.tile([128, 128], F32, tag="pa")
            nc.tensor.transpose(out=wt_ps[:], in_=W_sb[:], identity=ident[:])
            WT_sb = constp.tile([128, 128], F32)
            nc.vector.tensor_copy(out=WT_sb[:], in_=wt_ps[:])
            wlr_ps = pa_ps.tile([128, 2 * H], F32, tag="pa")
            nc.tensor.matmul(out=wlr_ps[:], lhsT=WT_sb[:], rhs=am_sb[:],
                             start=True, stop=True)
            Wh = constp.tile([128, HD + 2 * H], F16)
            nc.vector.tensor_copy(out=Wh[:, 0:HD], in_=W_sb[:])
            nc.vector.tensor_copy(out=Wh[:, HD:HD + 2 * H], in_=wlr_ps[:])

            # ---- residual x rows (slot-major) into SBUF ----
            xs_sb = bigp.tile([128, NBLK * HD], F16)
            nc.scalar.dma_start(
                out=xs_sb[:].rearrange("p (f c) -> p f c", c=HD),
                in_=xsd[:].rearrange("(f p) c -> p f c", f=NBLK))

            # ---- phase A: own table slice (x@W), er for own slots ----
            er_sb = bigp.tile([128, NBLK * H], F16)
            g4 = [(i * 4, min(4, NBLK - i * 4)) for i in range((NBLK + 3) // 4)]
            for (t0, nt) in g4:
                x4 = pa_sb.tile([128, 512], F16, tag="x4")
                nc.scalar.dma_start(out=x4[:, :nt * 128],
                                    in_=xTs[:, t0 * 128:(t0 + nt) * 128])
                xp4 = pa_sb.tile([128, 512], F16, tag="xp4")
                nc.sync.dma_start(out=xp4[:, :nt * 128],
                                  in_=xTp[:, t0 * 128:(t0 + nt) * 128])
                row4 = rowp.tile([128, 4 * HD], F16, tag="row4")
                er4_ps = pa_ps.tile([128, 4 * H], F32, tag="er4")
                for k in range(nt):
                    ps = pa_ps.tile([128, HD], F32, tag="pa")
                    nc.tensor.matmul(out=ps[:], lhsT=x4[:, k * 128:(k + 1) * 128],
                                     rhs=Wh[:, 0:HD], start=True, stop=True)
                    if k % 2 == 0:
                        nc.vector.tensor_copy(out=row4[:, k * HD:(k + 1) * HD],
                                              in_=ps[:])
                    else:
                        nc.scalar.activation(row4[:, k * HD:(k + 1) * HD],
                                             ps[:],
                                             mybir.ActivationFunctionType.Copy)
                    nc.tensor.matmul(out=er4_ps[:, k * H:(k + 1) * H],
                                     lhsT=xp4[:, k * 128:(k + 1) * 128],
                                     rhs=Wh[:, HD + H:HD + 2 * H],
                                     start=True, stop=True)
                nc.vector.tensor_copy(out=er_sb[:, t0 * H:(t0 + nt) * H],
                                      in_=er4_ps[:, 0:nt * H])
                nc.sync.dma_start(
                    out=tsrc[t0 * 128:(t0 + nt) * 128, :].rearrange(
                        "(f p) c -> p f c", f=nt),
                    in_=row4[:, 0:nt * HD].rearrange("p (f c) -> p f c", c=HD))
            pa_scope.__exit__(None, None, None)

            # ---- AllGather the table ----
            nc.gpsimd.collective_compute(
                kind="AllGather", op=mybir.AluOpType.bypass,
                replica_groups=RG, ins=[tsrc[:]], outs=[table[:]])

            # ---- phase B ----
            hall = bigp.tile([128, NBLK * HD], F32)

            sg_scope = tc.tile_pool(name="sg_ps", bufs=2, space="PSUM")
            sg_ps = sg_scope.__enter__()
            erp_scope = tc.tile_pool(name="erp_ps", bufs=1, space="PSUM")
            erp_psp = erp_scope.__enter__()
            psb_scope = tc.tile_pool(name="psb_ps", bufs=1, space="PSUM")
            psbp = psb_scope.__enter__()
            st_scope = tc.tile_pool(name="st_ps", bufs=1, space="PSUM")
            stat_ps = st_scope.__enter__()
            stat2_ps = stat_ps.tile([128, 2], F32)
            s1_ps = stat2_ps[:, 0:1]
            s2_ps = stat2_ps[:, 1:2]

            with (
                tc.tile_pool(name="gp", bufs=3) as gp,
                tc.tile_pool(name="gsp", bufs=2) as gsp,
                tc.tile_pool(name="idxp", bufs=2) as idxp,
                tc.tile_pool(name="drp", bufs=2) as drp,
                tc.tile_pool(name="st4p", bufs=2) as st4p,
                tc.tile_pool(name="ssp", bufs=2) as ssp,
                tc.tile_pool(name="tmpp", bufs=2) as tmpp,
                tc.tile_pool(name="elp", bufs=3) as elp,
                tc.tile_pool(name="wp", bufs=3) as wp,
                tc.tile_pool(name="finp", bufs=4) as finp,
            ):
                idx_col = 0   # running column offset into idxd (16 idxs/col)
                tile_col = 0  # running global tile index
                edge_off = 0  # running edge-slot offset (drow)
                gb_done = 0   # blocks finalized
                for (b0, nb) in SGS:
                    nt = nb * TPC              # tiles per (sg, q)
                    ne = nt * 128              # edge slots per (sg, q)
                    scols = ne // 16
                    idx_sb = idxp.tile([128, 4 * 160], I16, tag="idx")
                    nc.sync.dma_start(out=idx_sb[:, 0:4 * scols],
                                      in_=idxd[:, idx_col:idx_col + 4 * scols])
                    idx_col += 4 * scols
                    psbs = [psbp.tile([128, HD + H], F32, tag=f"psb{j}",
                                      name=f"psb{j}")
                            for j in range(nb)]
                    for q in range(4):
                        # --- batched gather of feat rows ---
                        g = gp.tile([128, 4 * TPC * 128], F16, tag="g")
                        nc.gpsimd.dma_gather(
                            g[:, 0:ne].rearrange("p (t c) -> p t c", c=128),
                            table[q * QROWS:(q + 1) * QROWS, :],
                            idx_sb[:, q * scols:(q + 1) * scols],
                            ne, ne, 128, single_packet=False, queue_num=q)
                        # --- el = reduce(feat * attn_l) ---
                        tmp = tmpp.tile([128, 4 * TPC * 128], F16, tag="tmp")
                        nc.vector.tensor_tensor(
                            out=tmp[:, 0:ne],
                            in0=g[:, 0:ne],
                            in1=attn_rep[:, 0:ne],
                            op=mybir.AluOpType.mult)
                        el = elp.tile([128, 4 * TPC * H], F32, tag="el")
                        nc.vector.tensor_reduce(
                            out=el[:, 0:nt * H],
                            in_=tmp[:, 0:ne].rearrange("p (s d) -> p s d", d=D),
                            axis=mybir.AxisListType.X,
                            op=mybir.AluOpType.add)
                        # --- st4 one-hot [slot, edge] for er expansion ---
                        dr = drp.tile([1, 4 * TPC * 128], F16, tag="dr")
                        nc.sync.dma_start(out=dr[:, 0:ne],
                                          in_=drd[:, edge_off:edge_off + ne])
                        edge_off += ne
                        st4 = st4p.tile([128, 4 * TPC * 128], F16, tag="st4")
                        for o in range(0, ne, 512):
                            cw = min(512, ne - o)
                            dtp = sg_ps.tile([128, 512], F32, tag="dtp")
                            nc.tensor.matmul(out=dtp[:, :cw], lhsT=ones_row[:],
                                             rhs=dr[:, o:o + cw],
                                             start=True, stop=True)
                            nc.vector.tensor_scalar(
                                out=st4[:, o:o + cw], in0=dtp[:, :cw],
                                scalar1=iota_col[:], scalar2=None,
                                op0=mybir.AluOpType.is_equal)
                        # --- er per edge via one-hot matmuls ---
                        erp = erp_psp.tile([128, 4 * TPC * H], F32, tag="erp")
                        for t in range(nt):
                            gb = b0 + t // TPC
                            nc.tensor.matmul(
                                out=erp[:, t * H:(t + 1) * H],
                                lhsT=st4[:, t * 128:(t + 1) * 128],
                                rhs=er_sb[:, gb * H:(gb + 1) * H],
                                start=True, stop=True)
                        # --- w = exp(leaky_relu(el + er)) ---
                        wb = wp.tile([128, 4 * TPC * H], F32, tag="wb")
                        nc.vector.tensor_tensor(out=wb[:, 0:nt * H],
                                                in0=el[:, 0:nt * H],
                                                in1=erp[:, 0:nt * H],
                                                op=mybir.AluOpType.add)
                        w5 = wp.tile([128, 4 * TPC * H], F32, tag="w5")
                        nc.vector.tensor_scalar(out=w5[:, 0:nt * H],
                                                in0=wb[:, 0:nt * H],
                                                scalar1=NEG_SLOPE, scalar2=None,
                                                op0=mybir.AluOpType.mult)
                        nc.vector.tensor_tensor(out=wb[:, 0:nt * H],
                                                in0=wb[:, 0:nt * H],
                                                in1=w5[:, 0:nt * H],
                                                op=mybir.AluOpType.max)
                        nc.scalar.activation(wb[:, 0:nt * H], wb[:, 0:nt * H],
                                             mybir.ActivationFunctionType.Exp)
                        # --- gs = [w*feat | w] fp16 ---
                        gs = gsp.tile([128, 4 * TPC * (HD + H)], F16, tag="gs")
                        gs_v = gs[:, 0:nt * (HD + H)].rearrange(
                            "p (t c) -> p t c", c=HD + H)
                        nc.scalar.activation(
                            gs_v[:, :, HD:HD + H],
                            wb[:, 0:nt * H],
                            mybir.ActivationFunctionType.Copy)
                        w16b = (gs_v[:, :, HD:HD + H]
                                .rearrange("p t (h o) -> p t h o", o=1)
                                .to_broadcast([128, nt, H, D]))
                        nc.vector.tensor_tensor(
                            out=gs_v[:, :, 0:HD].rearrange(
                                "p t (h d) -> p t h d", d=D),
                            in0=g[:, 0:ne].rearrange("p (t h d) -> p t h d",
                                                     h=H, d=D),
                            in1=w16b,
                            op=mybir.AluOpType.mult)
                        # --- s one-hot [edge, slot] + scatter matmuls ---
                        s_sb = ssp.tile([128, 4 * TPC * 128], F16, tag="s")
                        nc.vector.tensor_tensor(
                            out=s_sb[:, 0:ne].rearrange("p (t c) -> p t c",
                                                        c=128),
                            in0=iota_row[:].rearrange("p (o c) -> p o c", o=1)
                                .to_broadcast([128, nt, 128]),
                            in1=dslot_sb[:, tile_col + q * nt:
                                         tile_col + (q + 1) * nt]
                                .rearrange("p (t o) -> p t o", o=1)
                                .to_broadcast([128, nt, 128]),
                            op=mybir.AluOpType.is_equal)
                        for t in range(nt):
                            j = t // TPC
                            t5 = t % TPC
                            nc.tensor.matmul(
                                out=psbs[j][:],
                                lhsT=s_sb[:, t * 128:(t + 1) * 128],
                                rhs=gs[:, t * (HD + H):(t + 1) * (HD + H)],
                                start=(q == 0 and t5 == 0),
                                stop=(q == 3 and t5 == TPC - 1))
                    tile_col += 4 * nt
                    # --- finalize blocks of this supergroup ---
                    for j in range(nb):
                        gb = b0 + j
                        psb = psbs[j]
                        ssum = finp.tile([128, H], F32, tag="ssum")
                        nc.scalar.activation(ssum[:], psb[:, HD:HD + H],
                                             mybir.ActivationFunctionType.Copy,
                                             bias=1e-30)
                        rec = finp.tile([128, H], F32, tag="rec")
                        nc.vector.reciprocal(out=rec[:], in_=ssum[:])
                        rec_b = (rec[:].rearrange("p (h o) -> p h o", o=1)
                                 .to_broadcast([128, H, D]))
                        hslice = hall[:, gb * HD:(gb + 1) * HD]
                        nc.vector.tensor_tensor(
                            out=hslice.rearrange("p (h d) -> p h d", d=D),
                            in0=psb[:, 0:HD].rearrange("p (h d) -> p h d", d=D),
                            in1=rec_b, op=mybir.AluOpType.mult)
                        h16 = finp.tile([128, HD], F16, tag="h16")
                        nc.scalar.activation(h16[:], hslice,
                                             mybir.ActivationFunctionType.Copy)
                        sq16 = finp.tile([128, HD], F16, tag="sq16")
                        nc.scalar.activation(sq16[:], hslice,
                                             mybir.ActivationFunctionType.Square)
                        nc.tensor.matmul(out=s1_ps, lhsT=h16[:],
                                         rhs=ones_col16[:],
                                         start=(gb == 0), stop=(gb == NBLK - 1))
                        nc.tensor.matmul(out=s2_ps, lhsT=sq16[:],
                                         rhs=ones_col16[:],
                                         start=(gb == 0), stop=(gb == NBLK - 1))
                    gb_done += nb

            # ---- BN stats AllReduce + affine fold ----
            stat_sb = constp.tile([128, 2], F32)
            nc.vector.tensor_copy(out=stat_sb[:], in_=stat2_ps[:])
            nc.sync.dma_start(out=stat_in[:], in_=stat_sb[:])
            st_scope.__exit__(None, None, None)
            psb_scope.__exit__(None, None, None)
            erp_scope.__exit__(None, None, None)
            sg_scope.__exit__(None, None, None)
            nc.gpsimd.collective_compute(
                kind="AllReduce", op=mybir.AluOpType.add,
                replica_groups=RG, ins=[stat_in[:]], outs=[stat_out[:]])
            str_sb = constp.tile([128, 2], F32)
            nc.sync.dma_start(out=str_sb[:], in_=stat_out[:])

            ac2 = constp.tile([128, 2], F32)
            m1 = constp.tile([128, 1], F32)
            nc.vector.tensor_scalar(out=m1[:], in0=str_sb[:, 0:1],
                                    scalar1=1.0 / N, scalar2=None,
                                    op0=mybir.AluOpType.mult)
            m2 = constp.tile([128, 1], F32)
            nc.vector.tensor_scalar(out=m2[:], in0=str_sb[:, 1:2],
                                    scalar1=1.0 / N, scalar2=None,
                                    op0=mybir.AluOpType.mult)
            vv = constp.tile([128, 1], F32)
            nc.vector.tensor_tensor(out=vv[:], in0=m1[:], in1=m1[:],
                                    op=mybir.AluOpType.mult)
            nc.vector.tensor_tensor(out=vv[:], in0=m2[:], in1=vv[:],
                                    op=mybir.AluOpType.subtract)
            nc.vector.tensor_scalar(out=vv[:], in0=vv[:],
                                    scalar1=EPS, scalar2=None,
                                    op0=mybir.AluOpType.add)
            nc.scalar.activation(vv[:], vv[:],
                                 mybir.ActivationFunctionType.Sqrt)
            rsq = constp.tile([128, 1], F32)
            nc.vector.reciprocal(out=rsq[:], in_=vv[:])
            # a = gamma * rsqrt(var+eps)
            nc.vector.tensor_tensor(out=ac2[:, 0:1], in0=gbb_sb[:, 0:1],
                                    in1=rsq[:], op=mybir.AluOpType.mult)
            # c = beta - a * (mu + bias)
            mu = constp.tile([128, 1], F32)
            nc.vector.tensor_tensor(out=mu[:], in0=m1[:], in1=gbb_sb[:, 2:3],
                                    op=mybir.AluOpType.add)
            amu = constp.tile([128, 1], F32)
            nc.vector.tensor_tensor(out=amu[:], in0=ac2[:, 0:1], in1=mu[:],
                                    op=mybir.AluOpType.mult)
            nc.vector.tensor_tensor(out=ac2[:, 1:2], in0=gbb_sb[:, 1:2],
                                    in1=amu[:], op=mybir.AluOpType.subtract)

            fin_scope = tc.tile_pool(name="fin_ps", bufs=1, space="PSUM")
            fin_ps = fin_scope.__enter__()
            aT_ps = fin_ps.tile([1, 128], F32, tag="aT")
            nc.tensor.transpose(out=aT_ps[:], in_=ac2[:, 0:1],
                                identity=ident[:])
            cT_ps = fin_ps.tile([1, 128], F32, tag="cT")
            nc.tensor.transpose(out=cT_ps[:], in_=ac2[:, 1:2],
                                identity=ident[:])
            acT = constp.tile([1, 256], F32)
            nc.vector.tensor_copy(out=acT[:, 0:128], in_=aT_ps[:])
            nc.vector.tensor_copy(out=acT[:, 128:256], in_=cT_ps[:])
            ac_ps = fin_ps.tile([128, 256], F32, tag="AC")
            nc.tensor.matmul(out=ac_ps[:, 0:128], lhsT=ones_row32[:],
                             rhs=acT[0:1, 0:128], start=True, stop=True)
            nc.tensor.matmul(out=ac_ps[:, 128:256], lhsT=ones_row32[:],
                             rhs=acT[0:1, 128:256], start=True, stop=True)
            AC = constp.tile([128, 256], F32)
            nc.vector.tensor_copy(out=AC[:], in_=ac_ps[:])

            # ---- finalize: h2 = A*h + C ; ELU ; + x ; store ----
            with tc.tile_pool(name="fin2", bufs=3) as fin2p:
                FB = 8  # blocks per finalize group
                for f0 in range(0, NBLK, FB):
                    fb = min(FB, NBLK - f0)
                    w = fb * HD
                    h2 = fin2p.tile([128, FB * HD], F32, tag="h2")
                    nc.vector.tensor_tensor(
                        out=h2[:, 0:w].rearrange("p (f c) -> p f c", c=HD),
                        in0=hall[:, f0 * HD:(f0 + fb) * HD]
                            .rearrange("p (f c) -> p f c", c=HD),
                        in1=AC[:, 0:HD].rearrange("p (o c) -> p o c", o=1)
                            .to_broadcast([128, fb, HD]),
                        op=mybir.AluOpType.mult)
                    nc.vector.tensor_tensor(
                        out=h2[:, 0:w].rearrange("p (f c) -> p f c", c=HD),
                        in0=h2[:, 0:w].rearrange("p (f c) -> p f c", c=HD),
                        in1=AC[:, HD:256].rearrange("p (o c) -> p o c", o=1)
                            .to_broadcast([128, fb, HD]),
                        op=mybir.AluOpType.add)
                    m = fin2p.tile([128, FB * HD], F32, tag="m")
                    nc.vector.tensor_scalar(out=m[:, 0:w], in0=h2[:, 0:w],
                                            scalar1=0.0, scalar2=None,
                                            op0=mybir.AluOpType.min)
                    nc.scalar.activation(m[:, 0:w], m[:, 0:w],
                                         mybir.ActivationFunctionType.Exp)
                    nc.vector.tensor_scalar(out=m[:, 0:w], in0=m[:, 0:w],
                                            scalar1=-1.0, scalar2=None,
                                            op0=mybir.AluOpType.add)
                    nc.vector.tensor_tensor(out=h2[:, 0:w], in0=h2[:, 0:w],
                                            in1=m[:, 0:w],
                                            op=mybir.AluOpType.max)
                    nc.vector.tensor_tensor(
                        out=h2[:, 0:w], in0=h2[:, 0:w],
                        in1=xs_sb[:, f0 * HD:(f0 + fb) * HD],
                        op=mybir.AluOpType.add)
                    nc.sync.dma_start(
                        out=out_nm[f0 * 128:(f0 + fb) * 128, :].rearrange(
                            "(f p) c -> p f c", f=fb),
                        in_=h2[:, 0:w].rearrange("p (f c) -> p f c", c=HD))
            fin_scope.__exit__(None, None, None)

    nc.compile()
    return nc


def _host_prep(x, src, dst):
    """Shard + balance + pad. Returns per-core index arrays and mappings."""
    import heapq
    per_core = []
    equad_all = src // QROWS
    for c in range(NCORES):
        lo = c * SLOTS
        hi = min((c + 1) * SLOTS, N)
        nodes_c = hi - lo
        m = (dst >= lo) & (dst < hi)
        e_src = src[m].astype(np.int64)
        e_dst = (dst[m] - lo).astype(np.int64)
        e_q = equad_all[m].astype(np.int64)
        deg = np.bincount(e_dst, minlength=nodes_c)
        cq = np.zeros((nodes_c, 4), np.int64)
        np.add.at(cq, (e_dst, e_q), 1)
        order = np.argsort(-deg, kind="stable")
        # greedy balance with per-quadrant caps
        heap = [(0, b) for b in range(NBLK)]
        heapq.heapify(heap)
        slots_used = np.zeros(NBLK, np.int64)
        qload = np.zeros((NBLK, 4), np.int64)
        blk_of = np.empty(nodes_c, np.int64)
        slot_of = np.empty(nodes_c, np.int64)
        for v in order:
            spill = []
            while True:
                load, b = heapq.heappop(heap)
                if slots_used[b] < 128 and np.all(qload[b] + cq[v] <= QCAP):
                    break
                spill.append((load, b))
            blk_of[v] = b
            slot_of[v] = slots_used[b]
            slots_used[b] += 1
            qload[b] += cq[v]
            heapq.heappush(heap, (load + int(deg[v]), b))
            for it in spill:
                heapq.heappush(heap, it)
        # place edges into fixed (block, quadrant, 640) cells
        eb = blk_of[e_dst]
        eslot = slot_of[e_dst]
        key = eb * 4 + e_q
        cnt = np.bincount(key, minlength=NBLK * 4)
        assert cnt.max() <= QCAP, f"cell overflow {cnt.max()} > {QCAP}"
        eorder = np.argsort(key, kind="stable")
        offs = np.zeros(NBLK * 4 + 1, np.int64)
        np.cumsum(cnt, out=offs[1:])
        within = np.arange(len(eb)) - offs[key[eorder]]
        srcq_arr = np.zeros((NBLK, 4, QCAP), np.int16)
        slot_arr = np.full((NBLK, 4, QCAP), 300.0, np.float32)
        ko = key[eorder]
        srcq_arr[ko // 4, ko % 4, within] = (
            e_src[eorder] - e_q[eorder] * QROWS).astype(np.int16)
        slot_arr[ko // 4, ko % 4, within] = eslot[eorder]
        node_of_slot = np.full(SLOTS, -1, np.int64)
        node_of_slot[blk_of * 128 + slot_of] = np.arange(nodes_c) + lo
        per_core.append((srcq_arr, slot_arr, node_of_slot))
    return per_core


def _wrap16(vals):
    """int16 gather index layout: (p, s) = vals[s*16 + p%16], 8 replicas."""
    n = len(vals)
    w = vals.reshape(n // 16, 16).T.astype(np.int16)
    return np.tile(w, (8, 1))


def kernel(x, src, dst, W, attn_l, attn_r, bias, gamma, beta):
    global LAST_EXEC_NS
    x = np.asarray(x, np.float32)
    src = np.asarray(src, np.int64)
    dst = np.asarray(dst, np.int64)
    W = np.asarray(W, np.float32)
    attn_l = np.asarray(attn_l, np.float32)
    attn_r = np.asarray(attn_r, np.float32)
    bias = np.asarray(bias, np.float32)
    gamma = np.asarray(gamma, np.float32)
    beta = np.asarray(beta, np.float32)

    if "nc" not in _cache:
        _cache["nc"] = _build()
    nc = _cache["nc"]

    per_core = _host_prep(x, src, dst)

    xT = np.zeros((128, NTOT), np.float16)
    xT[:, :N] = x.T.astype(np.float16)
    am = np.zeros((HD, 2 * H), np.float32)
    for h in range(H):
        am[h * D:(h + 1) * D, h] = attn_l[h]
        am[h * D:(h + 1) * D, H + h] = attn_r[h]
    gbb = np.stack([gamma, beta, bias], axis=1).astype(np.float32)
    attnrep = np.tile(attn_l.reshape(1, H * D),
                      (128, 4 * TPC)).astype(np.float16)
    iota16 = np.tile(np.arange(128, dtype=np.float16), (128, 1))
    iotac = np.arange(128, dtype=np.float32).reshape(128, 1)

    in_maps = []
    for c in range(NCORES):
        srcq_arr, slot_arr, node_of_slot = per_core[c]
        # gather idx stream + dslot/drow in (sg, q, b, t, p) enumeration
        idx_cols = []
        ds_cols = []
        dr_vals = []
        for (b0, nb) in SGS:
            for q in range(4):
                flat_idx = srcq_arr[b0:b0 + nb, q, :].reshape(-1)
                flat_slot = slot_arr[b0:b0 + nb, q, :].reshape(-1)
                idx_cols.append(_wrap16(flat_idx))
                ds_cols.append(flat_slot.reshape(nb * TPC, 128).T)
                dr_vals.append(flat_slot)
        idx16 = np.concatenate(idx_cols, axis=1).astype(np.int16)
        dslot16 = np.concatenate(ds_cols, axis=1).astype(np.float16)
        drow = np.concatenate(dr_vals).reshape(1, EDGES_PAD).astype(np.float16)

        real = node_of_slot >= 0
        xTp = np.zeros((128, SLOTS), np.float16)
        xTp[:, real] = x[node_of_slot[real]].T.astype(np.float16)
        xs = np.zeros((SLOTS, HD), np.float16)
        xs[real] = x[node_of_slot[real]].astype(np.float16)

        in_maps.append({
            "xTs": np.ascontiguousarray(xT[:, c * SLOTS:(c + 1) * SLOTS]),
            "xTp": xTp, "xs": xs, "W": W, "am": am, "gbb": gbb,
            "attnrep": attnrep, "iota16": iota16, "iotac": iotac,
            "idx16": idx16, "dslot16": dslot16, "drow": drow,
        })

    res = run_bass_kernel_spmd(nc, in_maps, list(range(NCORES)),
                               **_trace_kwargs())
    LAST_EXEC_NS = [res.exec_time_ns or 0, 0]

    out = np.zeros((N, IN_DIM), np.float32)
    for c in range(NCORES):
        node_of_slot = per_core[c][2]
        real = node_of_slot >= 0
        om = res.results[c]["out_nm"]  # [SLOTS, HD]
        out[node_of_slot[real]] = om[real]
    return out


def _trace_kwargs():
    import os
    if os.environ.get("GAT_TRACE", "0") == "1":
        return {"trace": True}
    return {}


# revision 28
# speedup vs baseline: 1.4350x; 1.1591x over previous
"""GAT layer (DGL GATConv + BatchNorm + ELU + residual) on 8 Trainium2 cores.

Single-launch design (dst-sharded graph parallel):
  - Destination nodes load-balanced into 98 blocks x 128 slots per core.
  - Feat table [100352, 128] fp16 (256B rows) built distributed: each core
    computes its 1/8 slice (x @ W) and an AllGather replicates it.
  - Edge gathers use gpsimd dma_gather (batched SWDGE): one instruction per
    (supergroup of 4 blocks, src quadrant) fetches up to 2560 rows, killing
    the per-instruction ~1us Pool overhead that dominated the per-tile
    indirect-DMA baseline. int16 indices => 4 table windows of 25088 rows.
  - el per edge = reduce(feat * attn_l) on DVE; er per edge via one-hot
    matmuls (dst-slot one-hot built from iota is_equal).
  - Messages scaled by w = exp(leaky_relu(el+er)) in fp16; scatter-reduce
    into PSUM via S^T @ [w*feat | w] on the PE per 128-edge tile.
  - BatchNorm batch stats: per-core partial sums -> 1KB AllReduce -> affine
    fold + ELU + residual applied in-kernel (h stays in SBUF).
"""
import sys
sys.path.insert(0, "/opt/trn_rl_repo")
import numpy as np

import concourse.bass as bass
import concourse.bacc as bacc
import concourse.mybir as mybir
import concourse.tile as tile
from concourse.bass_utils import run_bass_kernel_spmd

F32 = mybir.dt.float32
F16 = mybir.dt.float16
I16 = mybir.dt.int16

N = 100000
E = 1600000
IN_DIM = 128
H = 8
D = 16
HD = 128
NCORES = 8
SLOTS = 12544             # dst slots per core (98 blocks x 128)
NBLK = 98
NTOT = NCORES * SLOTS     # 100352 padded node count
QROWS = NTOT // 4         # 25088 rows per int16-addressable table window
TPC = 5                   # tiles per (block, quadrant) cell
QCAP = TPC * 128          # 640 edge slots per cell
TILES = NBLK * 4 * TPC    # 1960 tiles per core
EDGES_PAD = TILES * 128   # 250880 edge slots per core
SGS = [(s * 4, min(4, NBLK - s * 4)) for s in range((NBLK + 3) // 4)]
NEG_SLOPE = 0.2
EPS = 1e-5

LAST_EXEC_NS = [0, 0]

_cache = {}


def _build():
    nc = bacc.Bacc("TRN2", target_bir_lowering=False, debug=False,
                   num_devices=NCORES, num_swdge_queues=4)
    xTs = nc.dram_tensor("xTs", [128, SLOTS], F16, kind="ExternalInput")
    xTp = nc.dram_tensor("xTp", [128, SLOTS], F16, kind="ExternalInput")
    xsd = nc.dram_tensor("xs", [SLOTS, HD], F16, kind="ExternalInput")
    Wd = nc.dram_tensor("W", [IN_DIM, HD], F32, kind="ExternalInput")
    amd = nc.dram_tensor("am", [HD, 2 * H], F32, kind="ExternalInput")
    gbbd = nc.dram_tensor("gbb", [128, 3], F32, kind="ExternalInput")
    atrd = nc.dram_tensor("attnrep", [128, 4 * TPC * 128], F16,
                          kind="ExternalInput")
    iotad = nc.dram_tensor("iota16", [128, 128], F16, kind="ExternalInput")
    iotacd = nc.dram_tensor("iotac", [128, 1], F32, kind="ExternalInput")
    idxd = nc.dram_tensor("idx16", [128, EDGES_PAD // 16], I16,
                          kind="ExternalInput")
    dsd = nc.dram_tensor("dslot16", [128, TILES], F16, kind="ExternalInput")

    out_nm = nc.dram_tensor("out_nm", [SLOTS, HD], F32, kind="ExternalOutput")
    table = nc.dram_tensor("table", [NTOT, HD], F16)
    tsrc = nc.dram_tensor("tsrc", [SLOTS, HD], F16)
    stat_in = nc.dram_tensor("stat_in", [128, 2], F32)
    stat_out = nc.dram_tensor("stat_out", [128, 2], F32)

    RG = [list(range(NCORES))]

    with tile.TileContext(nc) as tc:
        with (
            tc.tile_pool(name="const", bufs=1) as constp,
            tc.tile_pool(name="pa_sb", bufs=6) as pa_sb,
            tc.tile_pool(name="row4", bufs=4) as rowp,
            tc.tile_pool(name="big", bufs=1) as bigp,
        ):
            # ---- constants ----
            iota_row = constp.tile([128, 128], F16)
            nc.sync.dma_start(out=iota_row[:], in_=iotad[:])
            iota_col = constp.tile([128, 1], F32)
            nc.sync.dma_start(out=iota_col[:], in_=iotacd[:])
            attn_rep = constp.tile([128, 4 * TPC * 128], F16)
            nc.sync.dma_start(out=attn_rep[:], in_=atrd[:])
            gbb_sb = constp.tile([128, 3], F32)
            nc.sync.dma_start(out=gbb_sb[:], in_=gbbd[:])
            dslot_sb = constp.tile([128, TILES], F16)
            nc.sync.dma_start(out=dslot_sb[:], in_=dsd[:])
            ones_row = constp.tile([1, 128], F16)
            nc.vector.memset(ones_row[:], 1.0)
            ones_row32 = constp.tile([1, 128], F32)
            nc.vector.memset(ones_row32[:], 1.0)
            ones_col16 = constp.tile([128, 1], F16)
            nc.vector.memset(ones_col16[:], 1.0)

            from concourse.masks import make_identity
            ident = constp.tile([128, 128], F32)
            make_identity(nc, ident[:])
            ident16 = constp.tile([128, 128], F16)
            nc.vector.tensor_copy(out=ident16[:], in_=ident[:])

            # ---- Wh = [W | W@almat | W@armat] fp16 [128, 144] ----
            pa_scope = tc.tile_pool(name="pa_ps", bufs=4, space="PSUM")
            pa_ps = pa_scope.__enter__()
            W_sb = constp.tile([128, HD], F32)
            nc.sync.dma_start(out=W_sb[:], in_=Wd[:])
            am_sb = constp.tile([128, 2 * H], F32)
            nc.sync.dma_start(out=am_sb[:], in_=amd[:])
            wt_ps = pa_ps.tile([128, 128], F32, tag="pa")
            nc.tensor.transpose(out=wt_ps[:], in_=W_sb[:], identity=ident[:])
            WT_sb = constp.tile([128, 128], F32)
            nc.vector.tensor_copy(out=WT_sb[:], in_=wt_ps[:])
            wlr_ps = pa_ps.tile([128, 2 * H], F32, tag="pa")
            nc.tensor.matmul(out=wlr_ps[:], lhsT=WT_sb[:], rhs=am_sb[:],
                             start=True, stop=True)
            Wh = constp.tile([128, HD + 2 * H], F16)
            nc.vector.tensor_copy(out=Wh[:, 0:HD], in_=W_sb[:])
            nc.vector.tensor_copy(out=Wh[:, HD:HD + 2 * H], in_=wlr_ps[:])

            # ---- residual x rows (slot-major) into SBUF ----
            xs_sb = bigp.tile([128, NBLK * HD], F16)
            nc.scalar.dma_start(
                out=xs_sb[:].rearrange("p (f c) -> p f c", c=HD),
                in_=xsd[:].rearrange("(f p) c -> p f c", f=NBLK))

            # ---- phase A: own table slice (x@W), er for own slots ----
            er_sb = bigp.tile([128, NBLK * H], F16)
            g4 = [(i * 4, min(4, NBLK - i * 4)) for i in range((NBLK + 3) // 4)]
            for (t0, nt) in g4:
                x4 = pa_sb.tile([128, 512], F16, tag="x4")
                nc.scalar.dma_start(out=x4[:, :nt * 128],
                                    in_=xTs[:, t0 * 128:(t0 + nt) * 128])
                xp4 = pa_sb.tile([128, 512], F16, tag="xp4")
                nc.sync.dma_start(out=xp4[:, :nt * 128],
                                  in_=xTp[:, t0 * 128:(t0 + nt) * 128])
                row4 = rowp.tile([128, 4 * HD], F16, tag="row4")
                er4_ps = pa_ps.tile([128, 4 * H], F32, tag="er4")
                for k in range(nt):
                    ps = pa_ps.tile([128, HD], F32, tag="pa")
                    nc.tensor.matmul(out=ps[:], lhsT=x4[:, k * 128:(k + 1) * 128],
                                     rhs=Wh[:, 0:HD], start=True, stop=True)
                    if k % 2 == 0:
                        nc.vector.tensor_copy(out=row4[:, k * HD:(k + 1) * HD],
                                              in_=ps[:])
                    else:
                        nc.scalar.activation(row4[:, k * HD:(k + 1) * HD],
                                             ps[:],
                                             mybir.ActivationFunctionType.Copy)
                    nc.tensor.matmul(out=er4_ps[:, k * H:(k + 1) * H],
                                     lhsT=xp4[:, k * 128:(k + 1) * 128],
                                     rhs=Wh[:, HD + H:HD + 2 * H],
                                     start=True, stop=True)
                nc.vector.tensor_copy(out=er_sb[:, t0 * H:(t0 + nt) * H],
                                      in_=er4_ps[:, 0:nt * H])
                nc.sync.dma_start(
                    out=tsrc[t0 * 128:(t0 + nt) * 128, :].rearrange(
                        "(f p) c -> p f c", f=nt),
                    in_=row4[:, 0:nt * HD].rearrange("p (f c) -> p f c", c=HD))
            pa_scope.__exit__(None, None, None)

            # ---- AllGather the table ----
            nc.gpsimd.collective_compute(
                kind="AllGather", op=mybir.AluOpType.bypass,
                replica_groups=RG, ins=[tsrc[:]], outs=[table[:]])

            # ---- phase B ----
            hall = bigp.tile([128, NBLK * HD], F32)

            sg_scope = tc.tile_pool(name="tp_ps", bufs=2, space="PSUM")
            sg_ps = sg_scope.__enter__()
            erp_scope = tc.tile_pool(name="erp_ps", bufs=1, space="PSUM")
            erp_psp = erp_scope.__enter__()
            psb_scope = tc.tile_pool(name="psb_ps", bufs=1, space="PSUM")
            psbp = psb_scope.__enter__()
            st_scope = tc.tile_pool(name="st_ps", bufs=1, space="PSUM")
            stat_ps = st_scope.__enter__()
            stat2_ps = stat_ps.tile([128, 2], F32)
            s1_ps = stat2_ps[:, 0:1]
            s2_ps = stat2_ps[:, 1:2]

            with (
                tc.tile_pool(name="gp", bufs=3) as gp,
                tc.tile_pool(name="gsp", bufs=2) as gsp,
                tc.tile_pool(name="idxp", bufs=2) as idxp,
                tc.tile_pool(name="st4p", bufs=2) as st4p,
                tc.tile_pool(name="ssp", bufs=2) as ssp,
                tc.tile_pool(name="tmpp", bufs=2) as tmpp,
                tc.tile_pool(name="elp", bufs=3) as elp,
                tc.tile_pool(name="wp", bufs=3) as wp,
                tc.tile_pool(name="finp", bufs=4) as finp,
            ):
                idx_col = 0   # running column offset into idxd (16 idxs/col)
                tile_col = 0  # running global tile index
                gb_done = 0   # blocks finalized
                for (b0, nb) in SGS:
                    nt = nb * TPC              # tiles per (sg, q)
                    ne = nt * 128              # edge slots per (sg, q)
                    scols = ne // 16
                    idx_sb = idxp.tile([128, 4 * 160], I16, tag="idx")
                    nc.sync.dma_start(out=idx_sb[:, 0:4 * scols],
                                      in_=idxd[:, idx_col:idx_col + 4 * scols])
                    idx_col += 4 * scols
                    psbs = [psbp.tile([128, HD + H], F32, tag=f"psb{j}",
                                      name=f"psb{j}")
                            for j in range(nb)]
                    for q in range(4):
                        # --- batched gather of feat rows ---
                        g = gp.tile([128, 4 * TPC * 128], F16, tag="g")
                        nc.gpsimd.dma_gather(
                            g[:, 0:ne].rearrange("p (t c) -> p t c", c=128),
                            table[q * QROWS:(q + 1) * QROWS, :],
                            idx_sb[:, q * scols:(q + 1) * scols],
                            ne, ne, 128, single_packet=False, queue_num=q)
                        # --- el = reduce(feat * attn_l) ---
                        tmp = tmpp.tile([128, 4 * TPC * 128], F16, tag="tmp")
                        nc.vector.tensor_tensor(
                            out=tmp[:, 0:ne],
                            in0=g[:, 0:ne],
                            in1=attn_rep[:, 0:ne],
                            op=mybir.AluOpType.mult)
                        el = elp.tile([128, 4 * TPC * H], F32, tag="el")
                        nc.vector.tensor_reduce(
                            out=el[:, 0:nt * H],
                            in_=tmp[:, 0:ne].rearrange("p (s d) -> p s d", d=D),
                            axis=mybir.AxisListType.X,
                            op=mybir.AluOpType.add)
                        # --- s one-hot [edge, slot] (scatter lhsT) ---
                        s_sb = ssp.tile([128, 4 * TPC * 128], F16, tag="s")
                        nc.vector.tensor_tensor(
                            out=s_sb[:, 0:ne].rearrange("p (t c) -> p t c",
                                                        c=128),
                            in0=iota_row[:].rearrange("p (o c) -> p o c", o=1)
                                .to_broadcast([128, nt, 128]),
                            in1=dslot_sb[:, tile_col + q * nt:
                                         tile_col + (q + 1) * nt]
                                .rearrange("p (t o) -> p t o", o=1)
                                .to_broadcast([128, nt, 128]),
                            op=mybir.AluOpType.is_equal)
                        # --- st4 [slot, edge] = s^T via PE transpose ---
                        st4 = st4p.tile([128, 4 * TPC * 128], F16, tag="st4")
                        for t in range(nt):
                            tp = sg_ps.tile([128, 128], F16, tag="tp")
                            nc.tensor.transpose(
                                out=tp[:],
                                in_=s_sb[:, t * 128:(t + 1) * 128],
                                identity=ident16[:])
                            nc.scalar.activation(
                                st4[:, t * 128:(t + 1) * 128], tp[:],
                                mybir.ActivationFunctionType.Copy)
                        # --- er per edge via one-hot matmuls ---
                        erp = erp_psp.tile([128, 4 * TPC * H], F32, tag="erp")
                        for t in range(nt):
                            gb = b0 + t // TPC
                            nc.tensor.matmul(
                                out=erp[:, t * H:(t + 1) * H],
                                lhsT=st4[:, t * 128:(t + 1) * 128],
                                rhs=er_sb[:, gb * H:(gb + 1) * H],
                                start=True, stop=True)
                        # --- w = exp(leaky_relu(el + er)) ---
                        wb = wp.tile([128, 4 * TPC * H], F32, tag="wb")
                        nc.vector.tensor_tensor(out=wb[:, 0:nt * H],
                                                in0=el[:, 0:nt * H],
                                                in1=erp[:, 0:nt * H],
                                                op=mybir.AluOpType.add)
                        w5 = wp.tile([128, 4 * TPC * H], F32, tag="w5")
                        nc.vector.tensor_scalar(out=w5[:, 0:nt * H],
                                                in0=wb[:, 0:nt * H],
                                                scalar1=NEG_SLOPE, scalar2=None,
                                                op0=mybir.AluOpType.mult)
                        nc.vector.tensor_tensor(out=wb[:, 0:nt * H],
                                                in0=wb[:, 0:nt * H],
                                                in1=w5[:, 0:nt * H],
                                                op=mybir.AluOpType.max)
                        nc.scalar.activation(wb[:, 0:nt * H], wb[:, 0:nt * H],
                                             mybir.ActivationFunctionType.Exp)
                        # --- gs = [w*feat | w] fp16 ---
                        gs = gsp.tile([128, 4 * TPC * (HD + H)], F16, tag="gs")
                        gs_v = gs[:, 0:nt * (HD + H)].rearrange(
                            "p (t c) -> p t c", c=HD + H)
                        nc.scalar.activation(
                            gs_v[:, :, HD:HD + H],
                            wb[:, 0:nt * H],
                            mybir.ActivationFunctionType.Copy)
                        w16b = (gs_v[:, :, HD:HD + H]
                                .rearrange("p t (h o) -> p t h o", o=1)
                                .to_broadcast([128, nt, H, D]))
                        nc.vector.tensor_tensor(
                            out=gs_v[:, :, 0:HD].rearrange(
                                "p t (h d) -> p t h d", d=D),
                            in0=g[:, 0:ne].rearrange("p (t h d) -> p t h d",
                                                     h=H, d=D),
                            in1=w16b,
                            op=mybir.AluOpType.mult)
                        # --- scatter matmuls ---
                        for t in range(nt):
                            j = t // TPC
                            t5 = t % TPC
                            nc.tensor.matmul(
                                out=psbs[j][:],
                                lhsT=s_sb[:, t * 128:(t + 1) * 128],
                                rhs=gs[:, t * (HD + H):(t + 1) * (HD + H)],
                                start=(q == 0 and t5 == 0),
                                stop=(q == 3 and t5 == TPC - 1))
                    tile_col += 4 * nt
                    # --- finalize blocks of this supergroup ---
                    for j in range(nb):
                        gb = b0 + j
                        psb = psbs[j]
                        ssum = finp.tile([128, H], F32, tag="ssum")
                        nc.scalar.activation(ssum[:], psb[:, HD:HD + H],
                                             mybir.ActivationFunctionType.Copy,
                                             bias=1e-30)
                        rec = finp.tile([128, H], F32, tag="rec")
                        nc.vector.reciprocal(out=rec[:], in_=ssum[:])
                        rec_b = (rec[:].rearrange("p (h o) -> p h o", o=1)
                                 .to_broadcast([128, H, D]))
                        hslice = hall[:, gb * HD:(gb + 1) * HD]
                        nc.vector.tensor_tensor(
                            out=hslice.rearrange("p (h d) -> p h d", d=D),
                            in0=psb[:, 0:HD].rearrange("p (h d) -> p h d", d=D),
                            in1=rec_b, op=mybir.AluOpType.mult)
                        h16 = finp.tile([128, HD], F16, tag="h16")
                        nc.scalar.activation(h16[:], hslice,
                                             mybir.ActivationFunctionType.Copy)
                        sq16 = finp.tile([128, HD], F16, tag="sq16")
                        nc.scalar.activation(sq16[:], hslice,
                                             mybir.ActivationFunctionType.Square)
                        nc.tensor.matmul(out=s1_ps, lhsT=h16[:],
                                         rhs=ones_col16[:],
                                         start=(gb == 0), stop=(gb == NBLK - 1))
                        nc.tensor.matmul(out=s2_ps, lhsT=sq16[:],
                                         rhs=ones_col16[:],
                                         start=(gb == 0), stop=(gb == NBLK - 1))
                    gb_done += nb

            # ---- BN stats AllReduce + affine fold ----
            stat_sb = constp.tile([128, 2], F32)
            nc.vector.tensor_copy(out=stat_sb[:], in_=stat2_ps[:])
            nc.sync.dma_start(out=stat_in[:], in_=stat_sb[:])
            st_scope.__exit__(None, None, None)
            psb_scope.__exit__(None, None, None)
            erp_scope.__exit__(None, None, None)
            sg_scope.__exit__(None, None, None)
            nc.gpsimd.collective_compute(
                kind="AllReduce", op=mybir.AluOpType.add,
                replica_groups=RG, ins=[stat_in[:]], outs=[stat_out[:]])
            str_sb = constp.tile([128, 2], F32)
            nc.sync.dma_start(out=str_sb[:], in_=stat_out[:])

            ac2 = constp.tile([128, 2], F32)
            m1 = constp.tile([128, 1], F32)
            nc.vector.tensor_scalar(out=m1[:], in0=str_sb[:, 0:1],
                                    scalar1=1.0 / N, scalar2=None,
                                    op0=mybir.AluOpType.mult)
            m2 = constp.tile([128, 1], F32)
            nc.vector.tensor_scalar(out=m2[:], in0=str_sb[:, 1:2],
                                    scalar1=1.0 / N, scalar2=None,
                                    op0=mybir.AluOpType.mult)
            vv = constp.tile([128, 1], F32)
            nc.vector.tensor_tensor(out=vv[:], in0=m1[:], in1=m1[:],
                                    op=mybir.AluOpType.mult)
            nc.vector.tensor_tensor(out=vv[:], in0=m2[:], in1=vv[:],
                                    op=mybir.AluOpType.subtract)
            nc.vector.tensor_scalar(out=vv[:], in0=vv[:],
                                    scalar1=EPS, scalar2=None,
                                    op0=mybir.AluOpType.add)
            nc.scalar.activation(vv[:], vv[:],
                                 mybir.ActivationFunctionType.Sqrt)
            rsq = constp.tile([128, 1], F32)
            nc.vector.reciprocal(out=rsq[:], in_=vv[:])
            # a = gamma * rsqrt(var+eps)
            nc.vector.tensor_tensor(out=ac2[:, 0:1], in0=gbb_sb[:, 0:1],
                                    in1=rsq[:], op=mybir.AluOpType.mult)
            # c = beta - a * (mu + bias)
            mu = constp.tile([128, 1], F32)
            nc.vector.tensor_tensor(out=mu[:], in0=m1[:], in1=gbb_sb[:, 2:3],
                                    op=mybir.AluOpType.add)
            amu = constp.tile([128, 1], F32)
            nc.vector.tensor_tensor(out=amu[:], in0=ac2[:, 0:1], in1=mu[:],
                                    op=mybir.AluOpType.mult)
            nc.vector.tensor_tensor(out=ac2[:, 1:2], in0=gbb_sb[:, 1:2],
                                    in1=amu[:], op=mybir.AluOpType.subtract)

            fin_scope = tc.tile_pool(name="fin_ps", bufs=1, space="PSUM")
            fin_ps = fin_scope.__enter__()
            aT_ps = fin_ps.tile([1, 128], F32, tag="aT")
            nc.tensor.transpose(out=aT_ps[:], in_=ac2[:, 0:1],
                                identity=ident[:])
            cT_ps = fin_ps.tile([1, 128], F32, tag="cT")
            nc.tensor.transpose(out=cT_ps[:], in_=ac2[:, 1:2],
                                identity=ident[:])
            acT = constp.tile([1, 256], F32)
            nc.vector.tensor_copy(out=acT[:, 0:128], in_=aT_ps[:])
            nc.vector.tensor_copy(out=acT[:, 128:256], in_=cT_ps[:])
            ac_ps = fin_ps.tile([128, 256], F32, tag="AC")
            nc.tensor.matmul(out=ac_ps[:, 0:128], lhsT=ones_row32[:],
                             rhs=acT[0:1, 0:128], start=True, stop=True)
            nc.tensor.matmul(out=ac_ps[:, 128:256], lhsT=ones_row32[:],
                             rhs=acT[0:1, 128:256], start=True, stop=True)
            AC = constp.tile([128, 256], F32)
            nc.vector.tensor_copy(out=AC[:], in_=ac_ps[:])

            # ---- finalize: h2 = A*h + C ; ELU ; + x ; store ----
            with tc.tile_pool(name="fin2", bufs=3) as fin2p:
                FB = 8  # blocks per finalize group
                for f0 in range(0, NBLK, FB):
                    fb = min(FB, NBLK - f0)
                    w = fb * HD
                    h2 = fin2p.tile([128, FB * HD], F32, tag="h2")
                    nc.vector.tensor_tensor(
                        out=h2[:, 0:w].rearrange("p (f c) -> p f c", c=HD),
                        in0=hall[:, f0 * HD:(f0 + fb) * HD]
                            .rearrange("p (f c) -> p f c", c=HD),
                        in1=AC[:, 0:HD].rearrange("p (o c) -> p o c", o=1)
                            .to_broadcast([128, fb, HD]),
                        op=mybir.AluOpType.mult)
                    nc.vector.tensor_tensor(
                        out=h2[:, 0:w].rearrange("p (f c) -> p f c", c=HD),
                        in0=h2[:, 0:w].rearrange("p (f c) -> p f c", c=HD),
                        in1=AC[:, HD:256].rearrange("p (o c) -> p o c", o=1)
                            .to_broadcast([128, fb, HD]),
                        op=mybir.AluOpType.add)
                    m = fin2p.tile([128, FB * HD], F32, tag="m")
                    nc.vector.tensor_scalar(out=m[:, 0:w], in0=h2[:, 0:w],
                                            scalar1=0.0, scalar2=None,
                                            op0=mybir.AluOpType.min)
                    nc.scalar.activation(m[:, 0:w], m[:, 0:w],
                                         mybir.ActivationFunctionType.Exp)
                    nc.vector.tensor_scalar(out=m[:, 0:w], in0=m[:, 0:w],
                                            scalar1=-1.0, scalar2=None,
                                            op0=mybir.AluOpType.add)
                    nc.vector.tensor_tensor(out=h2[:, 0:w], in0=h2[:, 0:w],
                                            in1=m[:, 0:w],
                                            op=mybir.AluOpType.max)
                    nc.vector.tensor_tensor(
                        out=h2[:, 0:w], in0=h2[:, 0:w],
                        in1=xs_sb[:, f0 * HD:(f0 + fb) * HD],
                        op=mybir.AluOpType.add)
                    nc.sync.dma_start(
                        out=out_nm[f0 * 128:(f0 + fb) * 128, :].rearrange(
                            "(f p) c -> p f c", f=fb),
                        in_=h2[:, 0:w].rearrange("p (f c) -> p f c", c=HD))
            fin_scope.__exit__(None, None, None)

    nc.compile()
    return nc


def _host_prep(x, src, dst):
    """Shard + balance + pad. Returns per-core index arrays and mappings."""
    import heapq
    per_core = []
    equad_all = src // QROWS
    for c in range(NCORES):
        lo = c * SLOTS
        hi = min((c + 1) * SLOTS, N)
        nodes_c = hi - lo
        m = (dst >= lo) & (dst < hi)
        e_src = src[m].astype(np.int64)
        e_dst = (dst[m] - lo).astype(np.int64)
        e_q = equad_all[m].astype(np.int64)
        deg = np.bincount(e_dst, minlength=nodes_c)
        cq = np.zeros((nodes_c, 4), np.int64)
        np.add.at(cq, (e_dst, e_q), 1)
        order = np.argsort(-deg, kind="stable")
        # greedy balance with per-quadrant caps
        heap = [(0, b) for b in range(NBLK)]
        heapq.heapify(heap)
        slots_used = np.zeros(NBLK, np.int64)
        qload = np.zeros((NBLK, 4), np.int64)
        blk_of = np.empty(nodes_c, np.int64)
        slot_of = np.empty(nodes_c, np.int64)
        for v in order:
            spill = []
            while True:
                load, b = heapq.heappop(heap)
                if slots_used[b] < 128 and np.all(qload[b] + cq[v] <= QCAP):
                    break
                spill.append((load, b))
            blk_of[v] = b
            slot_of[v] = slots_used[b]
            slots_used[b] += 1
            qload[b] += cq[v]
            heapq.heappush(heap, (load + int(deg[v]), b))
            for it in spill:
                heapq.heappush(heap, it)
        # place edges into fixed (block, quadrant, 640) cells
        eb = blk_of[e_dst]
        eslot = slot_of[e_dst]
        key = eb * 4 + e_q
        cnt = np.bincount(key, minlength=NBLK * 4)
        assert cnt.max() <= QCAP, f"cell overflow {cnt.max()} > {QCAP}"
        eorder = np.argsort(key, kind="stable")
        offs = np.zeros(NBLK * 4 + 1, np.int64)
        np.cumsum(cnt, out=offs[1:])
        within = np.arange(len(eb)) - offs[key[eorder]]
        srcq_arr = np.zeros((NBLK, 4, QCAP), np.int16)
        slot_arr = np.full((NBLK, 4, QCAP), 300.0, np.float32)
        ko = key[eorder]
        srcq_arr[ko // 4, ko % 4, within] = (
            e_src[eorder] - e_q[eorder] * QROWS).astype(np.int16)
        slot_arr[ko // 4, ko % 4, within] = eslot[eorder]
        node_of_slot = np.full(SLOTS, -1, np.int64)
        node_of_slot[blk_of * 128 + slot_of] = np.arange(nodes_c) + lo
        per_core.append((srcq_arr, slot_arr, node_of_slot))
    return per_core


def _wrap16(vals):
    """int16 gather index layout: (p, s) = vals[s*16 + p%16], 8 replicas."""
    n = len(vals)
    w = vals.reshape(n // 16, 16).T.astype(np.int16)
    return np.tile(w, (8, 1))


def kernel(x, src, dst, W, attn_l, attn_r, bias, gamma, beta):
    global LAST_EXEC_NS
    x = np.asarray(x, np.float32)
    src = np.asarray(src, np.int64)
    dst = np.asarray(dst, np.int64)
    W = np.asarray(W, np.float32)
    attn_l = np.asarray(attn_l, np.float32)
    attn_r = np.asarray(attn_r, np.float32)
    bias = np.asarray(bias, np.float32)
    gamma = np.asarray(gamma, np.float32)
    beta = np.asarray(beta, np.float32)

    if "nc" not in _cache:
        _cache["nc"] = _build()
    nc = _cache["nc"]

    per_core = _host_prep(x, src, dst)

    xT = np.zeros((128, NTOT), np.float16)
    xT[:, :N] = x.T.astype(np.float16)
    am = np.zeros((HD, 2 * H), np.float32)
    for h in range(H):
        am[h * D:(h + 1) * D, h] = attn_l[h]
        am[h * D:(h + 1) * D, H + h] = attn_r[h]
    gbb = np.stack([gamma, beta, bias], axis=1).astype(np.float32)
    attnrep = np.tile(attn_l.reshape(1, H * D),
                      (128, 4 * TPC)).astype(np.float16)
    iota16 = np.tile(np.arange(128, dtype=np.float16), (128, 1))
    iotac = np.arange(128, dtype=np.float32).reshape(128, 1)

    in_maps = []
    for c in range(NCORES):
        srcq_arr, slot_arr, node_of_slot = per_core[c]
        # gather idx stream + dslot/drow in (sg, q, b, t, p) enumeration
        idx_cols = []
        ds_cols = []
        dr_vals = []
        for (b0, nb) in SGS:
            for q in range(4):
                flat_idx = srcq_arr[b0:b0 + nb, q, :].reshape(-1)
                flat_slot = slot_arr[b0:b0 + nb, q, :].reshape(-1)
                idx_cols.append(_wrap16(flat_idx))
                ds_cols.append(flat_slot.reshape(nb * TPC, 128).T)
                dr_vals.append(flat_slot)
        idx16 = np.concatenate(idx_cols, axis=1).astype(np.int16)
        dslot16 = np.concatenate(ds_cols, axis=1).astype(np.float16)

        real = node_of_slot >= 0
        xTp = np.zeros((128, SLOTS), np.float16)
        xTp[:, real] = x[node_of_slot[real]].T.astype(np.float16)
        xs = np.zeros((SLOTS, HD), np.float16)
        xs[real] = x[node_of_slot[real]].astype(np.float16)

        in_maps.append({
            "xTs": np.ascontiguousarray(xT[:, c * SLOTS:(c + 1) * SLOTS]),
            "xTp": xTp, "xs": xs, "W": W, "am": am, "gbb": gbb,
            "attnrep": attnrep, "iota16": iota16, "iotac": iotac,
            "idx16": idx16, "dslot16": dslot16,
        })

    res = run_bass_kernel_spmd(nc, in_maps, list(range(NCORES)),
                               **_trace_kwargs())
    LAST_EXEC_NS = [res.exec_time_ns or 0, 0]

    out = np.zeros((N, IN_DIM), np.float32)
    for c in range(NCORES):
        node_of_slot = per_core[c][2]
        real = node_of_slot >= 0
        om = res.results[c]["out_nm"]  # [SLOTS, HD]
        out[node_of_slot[real]] = om[real]
    return out


def _trace_kwargs():
    import os
    if os.environ.get("GAT_TRACE", "0") == "1":
        return {"trace": True}
    return {}


# revision 34
# speedup vs baseline: 1.4486x; 1.0095x over previous
"""GAT layer (DGL GATConv + BatchNorm + ELU + residual) on 8 Trainium2 cores.

Single-launch design (dst-sharded graph parallel):
  - Destination nodes load-balanced into 98 blocks x 128 slots per core.
  - Feat table [100352, 128] fp16 (256B rows) built distributed: each core
    computes its 1/8 slice (x @ W) and an AllGather replicates it.
  - Edge gathers use gpsimd dma_gather (batched SWDGE): one instruction per
    (supergroup of 4 blocks, src quadrant) fetches up to 2560 rows, killing
    the per-instruction ~1us Pool overhead that dominated the per-tile
    indirect-DMA baseline. int16 indices => 4 table windows of 25088 rows.
  - el per edge = reduce(feat * attn_l) on DVE; er per edge via one-hot
    matmuls (dst-slot one-hot built from iota is_equal).
  - Messages scaled by w = exp(leaky_relu(el+er)) in fp16; scatter-reduce
    into PSUM via S^T @ [w*feat | w] on the PE per 128-edge tile.
  - BatchNorm batch stats: per-core partial sums -> 1KB AllReduce -> affine
    fold + ELU + residual applied in-kernel (h stays in SBUF).
"""
import sys
sys.path.insert(0, "/opt/trn_rl_repo")
import numpy as np

import concourse.bass as bass
import concourse.bacc as bacc
import concourse.mybir as mybir
import concourse.tile as tile
from concourse.bass_utils import run_bass_kernel_spmd

F32 = mybir.dt.float32
F16 = mybir.dt.float16
I16 = mybir.dt.int16

N = 100000
E = 1600000
IN_DIM = 128
H = 8
D = 16
HD = 128
NCORES = 8
SLOTS = 12544             # dst slots per core (98 blocks x 128)
NBLK = 98
NTOT = NCORES * SLOTS     # 100352 padded node count
QROWS = NTOT // 4         # 25088 rows per int16-addressable table window
TPC = 5                   # tiles per (block, quadrant) cell
QCAP = TPC * 128          # 640 edge slots per cell
TILES = NBLK * 4 * TPC    # 1960 tiles per core
EDGES_PAD = TILES * 128   # 250880 edge slots per core
SGN = 2                   # blocks per supergroup
SGS = [(s * SGN, min(SGN, NBLK - s * SGN)) for s in range((NBLK + SGN - 1) // SGN)]
NEG_SLOPE = 0.2
EPS = 1e-5

LAST_EXEC_NS = [0, 0]

_cache = {}


def _build():
    nc = bacc.Bacc("TRN2", target_bir_lowering=False, debug=False,
                   num_devices=NCORES, num_swdge_queues=4)
    xTs = nc.dram_tensor("xTs", [128, SLOTS], F16, kind="ExternalInput")
    xTp = nc.dram_tensor("xTp", [128, SLOTS], F16, kind="ExternalInput")
    xsd = nc.dram_tensor("xs", [SLOTS, HD], F16, kind="ExternalInput")
    Wd = nc.dram_tensor("W", [IN_DIM, HD], F32, kind="ExternalInput")
    amd = nc.dram_tensor("am", [HD, 2 * H], F32, kind="ExternalInput")
    gbbd = nc.dram_tensor("gbb", [128, 3], F32, kind="ExternalInput")
    atrd = nc.dram_tensor("attnrep", [128, SGN * TPC * 128], F16,
                          kind="ExternalInput")
    iotad = nc.dram_tensor("iota16", [128, 128], F16, kind="ExternalInput")
    iotacd = nc.dram_tensor("iotac", [128, 1], F32, kind="ExternalInput")
    idxd = nc.dram_tensor("idx16", [128, EDGES_PAD // 16], I16,
                          kind="ExternalInput")
    dsd = nc.dram_tensor("dslot16", [128, TILES], F16, kind="ExternalInput")

    out_nm = nc.dram_tensor("out_nm", [SLOTS, HD], F32, kind="ExternalOutput")
    table = nc.dram_tensor("table", [NTOT, HD], F16)
    tsrc = nc.dram_tensor("tsrc", [SLOTS, HD], F16)
    stat_in = nc.dram_tensor("stat_in", [128, 2], F32)
    stat_out = nc.dram_tensor("stat_out", [128, 2], F32)

    RG = [list(range(NCORES))]

    with tile.TileContext(nc) as tc:
        with (
            tc.tile_pool(name="const", bufs=1) as constp,
            tc.tile_pool(name="pa_sb", bufs=6) as pa_sb,
            tc.tile_pool(name="row4", bufs=4) as rowp,
            tc.tile_pool(name="big", bufs=1) as bigp,
        ):
            # ---- constants ----
            iota_row = constp.tile([128, 128], F16)
            nc.sync.dma_start(out=iota_row[:], in_=iotad[:])
            iota_col = constp.tile([128, 1], F32)
            nc.sync.dma_start(out=iota_col[:], in_=iotacd[:])
            attn_rep = constp.tile([128, SGN * TPC * 128], F16)
            nc.sync.dma_start(out=attn_rep[:], in_=atrd[:])
            gbb_sb = constp.tile([128, 3], F32)
            nc.sync.dma_start(out=gbb_sb[:], in_=gbbd[:])
            dslot_sb = constp.tile([128, TILES], F16)
            nc.sync.dma_start(out=dslot_sb[:], in_=dsd[:])
            ones_row = constp.tile([1, 128], F16)
            nc.vector.memset(ones_row[:], 1.0)
            ones_row32 = constp.tile([1, 128], F32)
            nc.vector.memset(ones_row32[:], 1.0)
            ones_col16 = constp.tile([128, 1], F16)
            nc.vector.memset(ones_col16[:], 1.0)

            from concourse.masks import make_identity
            ident = constp.tile([128, 128], F32)
            make_identity(nc, ident[:])
            ident16 = constp.tile([128, 128], F16)
            nc.vector.tensor_copy(out=ident16[:], in_=ident[:])

            # ---- Wh = [W | W@almat | W@armat] fp16 [128, 144] ----
            pa_scope = tc.tile_pool(name="pa_ps", bufs=4, space="PSUM")
            pa_ps = pa_scope.__enter__()
            W_sb = constp.tile([128, HD], F32)
            nc.sync.dma_start(out=W_sb[:], in_=Wd[:])
            am_sb = constp.tile([128, 2 * H], F32)
            nc.sync.dma_start(out=am_sb[:], in_=amd[:])
            wt_ps = pa_ps.tile([128, 128], F32, tag="pa")
            nc.tensor.transpose(out=wt_ps[:], in_=W_sb[:], identity=ident[:])
            WT_sb = constp.tile([128, 128], F32)
            nc.vector.tensor_copy(out=WT_sb[:], in_=wt_ps[:])
            wlr_ps = pa_ps.tile([128, 2 * H], F32, tag="pa")
            nc.tensor.matmul(out=wlr_ps[:], lhsT=WT_sb[:], rhs=am_sb[:],
                             start=True, stop=True)
            Wh = constp.tile([128, HD + 2 * H], F16)
            nc.vector.tensor_copy(out=Wh[:, 0:HD], in_=W_sb[:])
            nc.vector.tensor_copy(out=Wh[:, HD:HD + 2 * H], in_=wlr_ps[:])

            # ---- residual x rows (slot-major) into SBUF ----
            xs_sb = bigp.tile([128, NBLK * HD], F16)
            nc.scalar.dma_start(
                out=xs_sb[:].rearrange("p (f c) -> p f c", c=HD),
                in_=xsd[:].rearrange("(f p) c -> p f c", f=NBLK))

            # ---- phase A: own table slice (x@W), er for own slots ----
            er_sb = bigp.tile([128, NBLK * H], F16)
            g4 = [(i * 4, min(4, NBLK - i * 4)) for i in range((NBLK + 3) // 4)]
            for (t0, nt) in g4:
                x4 = pa_sb.tile([128, 512], F16, tag="x4")
                nc.scalar.dma_start(out=x4[:, :nt * 128],
                                    in_=xTs[:, t0 * 128:(t0 + nt) * 128])
                xp4 = pa_sb.tile([128, 512], F16, tag="xp4")
                nc.sync.dma_start(out=xp4[:, :nt * 128],
                                  in_=xTp[:, t0 * 128:(t0 + nt) * 128])
                row4 = rowp.tile([128, 4 * HD], F16, tag="row4")
                er4_ps = pa_ps.tile([128, 4 * H], F32, tag="er4")
                for k in range(nt):
                    ps = pa_ps.tile([128, HD], F32, tag="pa")
                    nc.tensor.matmul(out=ps[:], lhsT=x4[:, k * 128:(k + 1) * 128],
                                     rhs=Wh[:, 0:HD], start=True, stop=True)
                    if k % 2 == 0:
                        nc.vector.tensor_copy(out=row4[:, k * HD:(k + 1) * HD],
                                              in_=ps[:])
                    else:
                        nc.scalar.activation(row4[:, k * HD:(k + 1) * HD],
                                             ps[:],
                                             mybir.ActivationFunctionType.Copy)
                    nc.tensor.matmul(out=er4_ps[:, k * H:(k + 1) * H],
                                     lhsT=xp4[:, k * 128:(k + 1) * 128],
                                     rhs=Wh[:, HD + H:HD + 2 * H],
                                     start=True, stop=True)
                nc.vector.tensor_copy(out=er_sb[:, t0 * H:(t0 + nt) * H],
                                      in_=er4_ps[:, 0:nt * H])
                nc.sync.dma_start(
                    out=tsrc[t0 * 128:(t0 + nt) * 128, :].rearrange(
                        "(f p) c -> p f c", f=nt),
                    in_=row4[:, 0:nt * HD].rearrange("p (f c) -> p f c", c=HD))
            pa_scope.__exit__(None, None, None)

            # ---- AllGather the table ----
            nc.gpsimd.collective_compute(
                kind="AllGather", op=mybir.AluOpType.bypass,
                replica_groups=RG, ins=[tsrc[:]], outs=[table[:]])

            # ---- phase B ----
            hall = bigp.tile([128, NBLK * HD], F32)

            sg_scope = tc.tile_pool(name="tp_ps", bufs=2, space="PSUM")
            sg_ps = sg_scope.__enter__()
            erp_scope = tc.tile_pool(name="erp_ps", bufs=1, space="PSUM")
            erp_psp = erp_scope.__enter__()
            psb_scope = tc.tile_pool(name="psb_ps", bufs=2, space="PSUM")
            psbp = psb_scope.__enter__()
            st_scope = tc.tile_pool(name="st_ps", bufs=1, space="PSUM")
            stat_ps = st_scope.__enter__()
            stat2_ps = stat_ps.tile([128, 2], F32)
            s1_ps = stat2_ps[:, 0:1]
            s2_ps = stat2_ps[:, 1:2]

            with (
                tc.tile_pool(name="gp", bufs=3) as gp,
                tc.tile_pool(name="gsp", bufs=2) as gsp,
                tc.tile_pool(name="idxp", bufs=2) as idxp,
                tc.tile_pool(name="st4p", bufs=2) as st4p,
                tc.tile_pool(name="ssp", bufs=2) as ssp,
                tc.tile_pool(name="tmpp", bufs=2) as tmpp,
                tc.tile_pool(name="elp", bufs=3) as elp,
                tc.tile_pool(name="wp", bufs=3) as wp,
                tc.tile_pool(name="finp", bufs=4) as finp,
            ):
                idx_col = 0   # running column offset into idxd (16 idxs/col)
                tile_col = 0  # running global tile index
                gb_done = 0   # blocks finalized
                for (b0, nb) in SGS:
                    nt = nb * TPC              # tiles per (sg, q)
                    ne = nt * 128              # edge slots per (sg, q)
                    scols = ne // 16
                    idx_sb = idxp.tile([128, 4 * SGN * 40], I16, tag="idx")
                    nc.sync.dma_start(out=idx_sb[:, 0:4 * scols],
                                      in_=idxd[:, idx_col:idx_col + 4 * scols])
                    idx_col += 4 * scols
                    psbs = [psbp.tile([128, HD + H], F32, tag=f"psb{j}",
                                      name=f"psb{j}")
                            for j in range(nb)]
                    for q in range(4):
                        # --- batched gather of feat rows ---
                        g = gp.tile([128, SGN * TPC * 128], F16, tag="g")
                        nc.gpsimd.dma_gather(
                            g[:, 0:ne].rearrange("p (t c) -> p t c", c=128),
                            table[q * QROWS:(q + 1) * QROWS, :],
                            idx_sb[:, q * scols:(q + 1) * scols],
                            ne, ne, 128, single_packet=False, queue_num=q)
                        # --- el = reduce(feat * attn_l) ---
                        tmp = tmpp.tile([128, SGN * TPC * 128], F16, tag="tmp")
                        nc.vector.tensor_tensor(
                            out=tmp[:, 0:ne],
                            in0=g[:, 0:ne],
                            in1=attn_rep[:, 0:ne],
                            op=mybir.AluOpType.mult)
                        el = elp.tile([128, SGN * TPC * H], F32, tag="el")
                        nc.vector.tensor_reduce(
                            out=el[:, 0:nt * H],
                            in_=tmp[:, 0:ne].rearrange("p (s d) -> p s d", d=D),
                            axis=mybir.AxisListType.X,
                            op=mybir.AluOpType.add)
                        # --- s one-hot [edge, slot] (scatter lhsT) ---
                        s_sb = ssp.tile([128, SGN * TPC * 128], F16, tag="s")
                        nc.vector.tensor_tensor(
                            out=s_sb[:, 0:ne].rearrange("p (t c) -> p t c",
                                                        c=128),
                            in0=iota_row[:].rearrange("p (o c) -> p o c", o=1)
                                .to_broadcast([128, nt, 128]),
                            in1=dslot_sb[:, tile_col + q * nt:
                                         tile_col + (q + 1) * nt]
                                .rearrange("p (t o) -> p t o", o=1)
                                .to_broadcast([128, nt, 128]),
                            op=mybir.AluOpType.is_equal)
                        # --- st4 [slot, edge] = s^T via PE transpose ---
                        st4 = st4p.tile([128, SGN * TPC * 128], F16, tag="st4")
                        for t in range(nt):
                            tp = sg_ps.tile([128, 128], F16, tag="tp")
                            nc.tensor.transpose(
                                out=tp[:],
                                in_=s_sb[:, t * 128:(t + 1) * 128],
                                identity=ident16[:])
                            nc.scalar.activation(
                                st4[:, t * 128:(t + 1) * 128], tp[:],
                                mybir.ActivationFunctionType.Copy)
                        # --- er per edge via one-hot matmuls ---
                        erp = erp_psp.tile([128, SGN * TPC * H], F32, tag="erp")
                        for t in range(nt):
                            gb = b0 + t // TPC
                            nc.tensor.matmul(
                                out=erp[:, t * H:(t + 1) * H],
                                lhsT=st4[:, t * 128:(t + 1) * 128],
                                rhs=er_sb[:, gb * H:(gb + 1) * H],
                                start=True, stop=True)
                        # --- w = exp(leaky_relu(el + er)) ---
                        wb = wp.tile([128, SGN * TPC * H], F32, tag="wb")
                        nc.vector.tensor_tensor(out=wb[:, 0:nt * H],
                                                in0=el[:, 0:nt * H],
                                                in1=erp[:, 0:nt * H],
                                                op=mybir.AluOpType.add)
                        w5 = wp.tile([128, SGN * TPC * H], F32, tag="w5")
                        nc.vector.tensor_scalar(out=w5[:, 0:nt * H],
                                                in0=wb[:, 0:nt * H],
                                                scalar1=NEG_SLOPE, scalar2=None,
                                                op0=mybir.AluOpType.mult)
                        nc.vector.tensor_tensor(out=wb[:, 0:nt * H],
                                                in0=wb[:, 0:nt * H],
                                                in1=w5[:, 0:nt * H],
                                                op=mybir.AluOpType.max)
                        nc.scalar.activation(wb[:, 0:nt * H], wb[:, 0:nt * H],
                                             mybir.ActivationFunctionType.Exp)
                        # --- gs = [w*feat | w] fp16 ---
                        gs = gsp.tile([128, SGN * TPC * (HD + H)], F16, tag="gs")
                        gs_v = gs[:, 0:nt * (HD + H)].rearrange(
                            "p (t c) -> p t c", c=HD + H)
                        nc.scalar.activation(
                            gs_v[:, :, HD:HD + H],
                            wb[:, 0:nt * H],
                            mybir.ActivationFunctionType.Copy)
                        w16b = (gs_v[:, :, HD:HD + H]
                                .rearrange("p t (h o) -> p t h o", o=1)
                                .to_broadcast([128, nt, H, D]))
                        nc.vector.tensor_tensor(
                            out=gs_v[:, :, 0:HD].rearrange(
                                "p t (h d) -> p t h d", d=D),
                            in0=g[:, 0:ne].rearrange("p (t h d) -> p t h d",
                                                     h=H, d=D),
                            in1=w16b,
                            op=mybir.AluOpType.mult)
                        # --- scatter matmuls ---
                        for t in range(nt):
                            j = t // TPC
                            t5 = t % TPC
                            nc.tensor.matmul(
                                out=psbs[j][:],
                                lhsT=s_sb[:, t * 128:(t + 1) * 128],
                                rhs=gs[:, t * (HD + H):(t + 1) * (HD + H)],
                                start=(q == 0 and t5 == 0),
                                stop=(q == 3 and t5 == TPC - 1))
                    tile_col += 4 * nt
                    # --- finalize blocks of this supergroup ---
                    for j in range(nb):
                        gb = b0 + j
                        psb = psbs[j]
                        ssum = finp.tile([128, H], F32, tag="ssum")
                        nc.scalar.activation(ssum[:], psb[:, HD:HD + H],
                                             mybir.ActivationFunctionType.Copy,
                                             bias=1e-30)
                        rec = finp.tile([128, H], F32, tag="rec")
                        nc.vector.reciprocal(out=rec[:], in_=ssum[:])
                        rec_b = (rec[:].rearrange("p (h o) -> p h o", o=1)
                                 .to_broadcast([128, H, D]))
                        hslice = hall[:, gb * HD:(gb + 1) * HD]
                        nc.vector.tensor_tensor(
                            out=hslice.rearrange("p (h d) -> p h d", d=D),
                            in0=psb[:, 0:HD].rearrange("p (h d) -> p h d", d=D),
                            in1=rec_b, op=mybir.AluOpType.mult)
                        h16 = finp.tile([128, HD], F16, tag="h16")
                        nc.scalar.activation(h16[:], hslice,
                                             mybir.ActivationFunctionType.Copy)
                        sq16 = finp.tile([128, HD], F16, tag="sq16")
                        nc.scalar.activation(sq16[:], hslice,
                                             mybir.ActivationFunctionType.Square)
                        nc.tensor.matmul(out=s1_ps, lhsT=h16[:],
                                         rhs=ones_col16[:],
                                         start=(gb == 0), stop=(gb == NBLK - 1))
                        nc.tensor.matmul(out=s2_ps, lhsT=sq16[:],
                                         rhs=ones_col16[:],
                                         start=(gb == 0), stop=(gb == NBLK - 1))
                    gb_done += nb

            # ---- BN stats AllReduce + affine fold ----
            stat_sb = constp.tile([128, 2], F32)
            nc.vector.tensor_copy(out=stat_sb[:], in_=stat2_ps[:])
            nc.sync.dma_start(out=stat_in[:], in_=stat_sb[:])
            st_scope.__exit__(None, None, None)
            psb_scope.__exit__(None, None, None)
            erp_scope.__exit__(None, None, None)
            sg_scope.__exit__(None, None, None)
            nc.gpsimd.collective_compute(
                kind="AllReduce", op=mybir.AluOpType.add,
                replica_groups=RG, ins=[stat_in[:]], outs=[stat_out[:]])
            str_sb = constp.tile([128, 2], F32)
            nc.sync.dma_start(out=str_sb[:], in_=stat_out[:])

            ac2 = constp.tile([128, 2], F32)
            m1 = constp.tile([128, 1], F32)
            nc.vector.tensor_scalar(out=m1[:], in0=str_sb[:, 0:1],
                                    scalar1=1.0 / N, scalar2=None,
                                    op0=mybir.AluOpType.mult)
            m2 = constp.tile([128, 1], F32)
            nc.vector.tensor_scalar(out=m2[:], in0=str_sb[:, 1:2],
                                    scalar1=1.0 / N, scalar2=None,
                                    op0=mybir.AluOpType.mult)
            vv = constp.tile([128, 1], F32)
            nc.vector.tensor_tensor(out=vv[:], in0=m1[:], in1=m1[:],
                                    op=mybir.AluOpType.mult)
            nc.vector.tensor_tensor(out=vv[:], in0=m2[:], in1=vv[:],
                                    op=mybir.AluOpType.subtract)
            nc.vector.tensor_scalar(out=vv[:], in0=vv[:],
                                    scalar1=EPS, scalar2=None,
                                    op0=mybir.AluOpType.add)
            nc.scalar.activation(vv[:], vv[:],
                                 mybir.ActivationFunctionType.Sqrt)
            rsq = constp.tile([128, 1], F32)
            nc.vector.reciprocal(out=rsq[:], in_=vv[:])
            # a = gamma * rsqrt(var+eps)
            nc.vector.tensor_tensor(out=ac2[:, 0:1], in0=gbb_sb[:, 0:1],
                                    in1=rsq[:], op=mybir.AluOpType.mult)
            # c = beta - a * (mu + bias)
            mu = constp.tile([128, 1], F32)
            nc.vector.tensor_tensor(out=mu[:], in0=m1[:], in1=gbb_sb[:, 2:3],
                                    op=mybir.AluOpType.add)
            amu = constp.tile([128, 1], F32)
            nc.vector.tensor_tensor(out=amu[:], in0=ac2[:, 0:1], in1=mu[:],
                                    op=mybir.AluOpType.mult)
            nc.vector.tensor_tensor(out=ac2[:, 1:2], in0=gbb_sb[:, 1:2],
                                    in1=amu[:], op=mybir.AluOpType.subtract)

            fin_scope = tc.tile_pool(name="fin_ps", bufs=1, space="PSUM")
            fin_ps = fin_scope.__enter__()
            aT_ps = fin_ps.tile([1, 128], F32, tag="aT")
            nc.tensor.transpose(out=aT_ps[:], in_=ac2[:, 0:1],
                                identity=ident[:])
            cT_ps = fin_ps.tile([1, 128], F32, tag="cT")
            nc.tensor.transpose(out=cT_ps[:], in_=ac2[:, 1:2],
                                identity=ident[:])
            acT = constp.tile([1, 256], F32)
            nc.vector.tensor_copy(out=acT[:, 0:128], in_=aT_ps[:])
            nc.vector.tensor_copy(out=acT[:, 128:256], in_=cT_ps[:])
            ac_ps = fin_ps.tile([128, 256], F32, tag="AC")
            nc.tensor.matmul(out=ac_ps[:, 0:128], lhsT=ones_row32[:],
                             rhs=acT[0:1, 0:128], start=True, stop=True)
            nc.tensor.matmul(out=ac_ps[:, 128:256], lhsT=ones_row32[:],
                             rhs=acT[0:1, 128:256], start=True, stop=True)
            AC = constp.tile([128, 256], F32)
            nc.vector.tensor_copy(out=AC[:], in_=ac_ps[:])

            # ---- finalize: h2 = A*h + C ; ELU ; + x ; store ----
            with tc.tile_pool(name="fin2", bufs=3) as fin2p:
                FB = 8  # blocks per finalize group
                for f0 in range(0, NBLK, FB):
                    fb = min(FB, NBLK - f0)
                    w = fb * HD
                    h2 = fin2p.tile([128, FB * HD], F32, tag="h2")
                    nc.vector.tensor_tensor(
                        out=h2[:, 0:w].rearrange("p (f c) -> p f c", c=HD),
                        in0=hall[:, f0 * HD:(f0 + fb) * HD]
                            .rearrange("p (f c) -> p f c", c=HD),
                        in1=AC[:, 0:HD].rearrange("p (o c) -> p o c", o=1)
                            .to_broadcast([128, fb, HD]),
                        op=mybir.AluOpType.mult)
                    nc.vector.tensor_tensor(
                        out=h2[:, 0:w].rearrange("p (f c) -> p f c", c=HD),
                        in0=h2[:, 0:w].rearrange("p (f c) -> p f c", c=HD),
                        in1=AC[:, HD:256].rearrange("p (o c) -> p o c", o=1)
                            .to_broadcast([128, fb, HD]),
                        op=mybir.AluOpType.add)
                    m = fin2p.tile([128, FB * HD], F32, tag="m")
                    nc.vector.tensor_scalar(out=m[:, 0:w], in0=h2[:, 0:w],
                                            scalar1=0.0, scalar2=None,
                                            op0=mybir.AluOpType.min)
                    nc.scalar.activation(m[:, 0:w], m[:, 0:w],
                                         mybir.ActivationFunctionType.Exp)
                    nc.vector.tensor_scalar(out=m[:, 0:w], in0=m[:, 0:w],
                                            scalar1=-1.0, scalar2=None,
                                            op0=mybir.AluOpType.add)
                    nc.vector.tensor_tensor(out=h2[:, 0:w], in0=h2[:, 0:w],
                                            in1=m[:, 0:w],
                                            op=mybir.AluOpType.max)
                    nc.vector.tensor_tensor(
                        out=h2[:, 0:w], in0=h2[:, 0:w],
                        in1=xs_sb[:, f0 * HD:(f0 + fb) * HD],
                        op=mybir.AluOpType.add)
                    nc.sync.dma_start(
                        out=out_nm[f0 * 128:(f0 + fb) * 128, :].rearrange(
                            "(f p) c -> p f c", f=fb),
                        in_=h2[:, 0:w].rearrange("p (f c) -> p f c", c=HD))
            fin_scope.__exit__(None, None, None)

    nc.compile()
    return nc


def _host_prep(x, src, dst):
    """Shard + balance + pad. Returns per-core index arrays and mappings."""
    import heapq
    per_core = []
    equad_all = src // QROWS
    for c in range(NCORES):
        lo = c * SLOTS
        hi = min((c + 1) * SLOTS, N)
        nodes_c = hi - lo
        m = (dst >= lo) & (dst < hi)
        e_src = src[m].astype(np.int64)
        e_dst = (dst[m] - lo).astype(np.int64)
        e_q = equad_all[m].astype(np.int64)
        deg = np.bincount(e_dst, minlength=nodes_c)
        cq = np.zeros((nodes_c, 4), np.int64)
        np.add.at(cq, (e_dst, e_q), 1)
        order = np.argsort(-deg, kind="stable")
        # greedy balance with per-quadrant caps
        heap = [(0, b) for b in range(NBLK)]
        heapq.heapify(heap)
        slots_used = np.zeros(NBLK, np.int64)
        qload = np.zeros((NBLK, 4), np.int64)
        blk_of = np.empty(nodes_c, np.int64)
        slot_of = np.empty(nodes_c, np.int64)
        for v in order:
            spill = []
            while True:
                load, b = heapq.heappop(heap)
                if slots_used[b] < 128 and np.all(qload[b] + cq[v] <= QCAP):
                    break
                spill.append((load, b))
            blk_of[v] = b
            slot_of[v] = slots_used[b]
            slots_used[b] += 1
            qload[b] += cq[v]
            heapq.heappush(heap, (load + int(deg[v]), b))
            for it in spill:
                heapq.heappush(heap, it)
        # place edges into fixed (block, quadrant, 640) cells
        eb = blk_of[e_dst]
        eslot = slot_of[e_dst]
        key = eb * 4 + e_q
        cnt = np.bincount(key, minlength=NBLK * 4)
        assert cnt.max() <= QCAP, f"cell overflow {cnt.max()} > {QCAP}"
        eorder = np.argsort(key, kind="stable")
        offs = np.zeros(NBLK * 4 + 1, np.int64)
        np.cumsum(cnt, out=offs[1:])
        within = np.arange(len(eb)) - offs[key[eorder]]
        srcq_arr = np.zeros((NBLK, 4, QCAP), np.int16)
        slot_arr = np.full((NBLK, 4, QCAP), 300.0, np.float32)
        ko = key[eorder]
        srcq_arr[ko // 4, ko % 4, within] = (
            e_src[eorder] - e_q[eorder] * QROWS).astype(np.int16)
        slot_arr[ko // 4, ko % 4, within] = eslot[eorder]
        node_of_slot = np.full(SLOTS, -1, np.int64)
        node_of_slot[blk_of * 128 + slot_of] = np.arange(nodes_c) + lo
        per_core.append((srcq_arr, slot_arr, node_of_slot))
    return per_core


def _wrap16(vals):
    """int16 gather index layout: (p, s) = vals[s*16 + p%16], 8 replicas."""
    n = len(vals)
    w = vals.reshape(n // 16, 16).T.astype(np.int16)
    return np.tile(w, (8, 1))


def kernel(x, src, dst, W, attn_l, attn_r, bias, gamma, beta):
    global LAST_EXEC_NS
    x = np.asarray(x, np.float32)
    src = np.asarray(src, np.int64)
    dst = np.asarray(dst, np.int64)
    W = np.asarray(W, np.float32)
    attn_l = np.asarray(attn_l, np.float32)
    attn_r = np.asarray(attn_r, np.float32)
    bias = np.asarray(bias, np.float32)
    gamma = np.asarray(gamma, np.float32)
    beta = np.asarray(beta, np.float32)

    if "nc" not in _cache:
        _cache["nc"] = _build()
    nc = _cache["nc"]

    per_core = _host_prep(x, src, dst)

    xT = np.zeros((128, NTOT), np.float16)
    xT[:, :N] = x.T.astype(np.float16)
    am = np.zeros((HD, 2 * H), np.float32)
    for h in range(H):
        am[h * D:(h + 1) * D, h] = attn_l[h]
        am[h * D:(h + 1) * D, H + h] = attn_r[h]
    gbb = np.stack([gamma, beta, bias], axis=1).astype(np.float32)
    attnrep = np.tile(attn_l.reshape(1, H * D),
                      (128, SGN * TPC)).astype(np.float16)
    iota16 = np.tile(np.arange(128, dtype=np.float16), (128, 1))
    iotac = np.arange(128, dtype=np.float32).reshape(128, 1)

    in_maps = []
    for c in range(NCORES):
        srcq_arr, slot_arr, node_of_slot = per_core[c]
        # gather idx stream + dslot/drow in (sg, q, b, t, p) enumeration
        idx_cols = []
        ds_cols = []
        dr_vals = []
        for (b0, nb) in SGS:
            for q in range(4):
                flat_idx = srcq_arr[b0:b0 + nb, q, :].reshape(-1)
                flat_slot = slot_arr[b0:b0 + nb, q, :].reshape(-1)
                idx_cols.append(_wrap16(flat_idx))
                ds_cols.append(flat_slot.reshape(nb * TPC, 128).T)
                dr_vals.append(flat_slot)
        idx16 = np.concatenate(idx_cols, axis=1).astype(np.int16)
        dslot16 = np.concatenate(ds_cols, axis=1).astype(np.float16)

        real = node_of_slot >= 0
        xTp = np.zeros((128, SLOTS), np.float16)
        xTp[:, real] = x[node_of_slot[real]].T.astype(np.float16)
        xs = np.zeros((SLOTS, HD), np.float16)
        xs[real] = x[node_of_slot[real]].astype(np.float16)

        in_maps.append({
            "xTs": np.ascontiguousarray(xT[:, c * SLOTS:(c + 1) * SLOTS]),
            "xTp": xTp, "xs": xs, "W": W, "am": am, "gbb": gbb,
            "attnrep": attnrep, "iota16": iota16, "iotac": iotac,
            "idx16": idx16, "dslot16": dslot16,
        })

    res = run_bass_kernel_spmd(nc, in_maps, list(range(NCORES)),
                               **_trace_kwargs())
    LAST_EXEC_NS = [res.exec_time_ns or 0, 0]

    out = np.zeros((N, IN_DIM), np.float32)
    for c in range(NCORES):
        node_of_slot = per_core[c][2]
        real = node_of_slot >= 0
        om = res.results[c]["out_nm"]  # [SLOTS, HD]
        out[node_of_slot[real]] = om[real]
    return out


def _trace_kwargs():
    import os
    if os.environ.get("GAT_TRACE", "0") == "1":
        return {"trace": True}
    return {}
